# revision 74
# baseline (speedup 1.0000x reference)
"""Trainium2 Bass kernel for AtomInteractionWithResidual (PhysNet-style GNN block).

Strategy (8 NeuronCores, data-parallel over batch B=32 -> 4 batches/core):
  - Host-side prep (layout only): rbf transposed to [K, edges] (bf16),
    MLP weights pre-transposed to lhsT form, softplus-shift (ln 2) folded
    into biases.
  - Neighbor-feature materialization is SPLIT per batch to balance the DMA
    engines against the PE array:
      * edges 0..9216   (T_G=72 tiles): dma_gather of softplus(x) rows from
        DRAM (bf16, 2 gather calls per batch)
      * edges 9216..16384 (T_H=56 tiles): one-hot matmul on the PE --
        xj_tile = H_tile.T @ xash where H is a host-built fp8 one-hot
        (exact 0/1 selection, 2 accumulating passes over the 256-atom
        contraction). H streams in as contiguous fp8 DMA, which is ~2x
        cheaper per edge than per-row gather descriptors. The resulting
        PSUM xj is evacuated to SBUF bf16 on the Act engine (DVE can only
        read one PSUM operand).
  - Device per batch:
      xa = softplus(x) - ln2 (bf16) -> DRAM (gather source) + SBUF (H rhs)
      g  = rbfT.T @ k2fT per 128-edge tile (PE)        [edge, F] f32 PSUM
      u  = g * xj (DVE)                                [edge, F] bf16
      sT += sel-window.T @ u (PE, accumulating)        [F, n]    feature-major
      feature-major MLP chain (interaction res blocks, gate, atom res blocks)
      using float32r matmuls (4x faster than f32 on the PE at 256-wide).
  - Scheduling structure (the cost model serializes all DMA on one device
    and every engine queue is in-order):
      * x->xa prologue for all batches runs first; compute-dependent writes
        (xa, out) issue on the Activation engine's HWDGE queue so they never
        head-of-line block the SP bulk-load queue.
      * PSUM pools use batch-parity tags (sT0/sT1, z0/z1) so adjacent
        batches' serial MLP chains get disjoint banks and can overlap.
      * Each batch's MLP chain is emitted as a generator advanced between
        the next batch's aggregation groups (program-order interleaving).
      * Batch 0 processes one-hot groups first (their inputs need no
        DRAM round-trip), starting the PE/DVE pipeline ~15us earlier.
"""

import numpy as np
import ml_dtypes
from contextlib import ExitStack

import concourse.bass as bass
from concourse import bacc
import concourse.mybir as mybir
import concourse.tile as tile
from concourse.bass_utils import run_bass_kernel_spmd

F32 = mybir.dt.float32
F32R = mybir.dt.float32r
BF16 = mybir.dt.bfloat16
FP8 = mybir.dt.float8e4
I16 = mybir.dt.int16
AF = mybir.ActivationFunctionType
ALU = mybir.AluOpType

B, N, M, F, K = 32, 256, 64, 128, 64
NCORES = 8
BPC = B // NCORES          # batches per core
E_B = N * M                # edges per batch (16384)
ET_B = E_B // 128          # 128-edge tiles per batch (128)
T_G = 56                   # tiles materialized via dma_gather
T_H = ET_B - T_G           # tiles materialized via one-hot matmul (72)
E_G = T_G * 128            # gathered edges per batch (7168)
E_H = T_H * 128            # one-hot edges per batch (9216)
G_SIZES = [E_G // 2, E_G // 2]  # gather call sizes
assert sum(G_SIZES) == E_G
GRP_SPLIT = T_G // 4       # group index where the H-sourced tiles start (14)
LN2 = float(np.log(2.0))

# weight stack order (lhsT = W.T each)
IW_WI = 0
IW_IRES = 1                # 1..6: (W1,W2) x 3
IW_WINT = 7
IW_ARES = 8                # 8..11: (W1,W2) x 2
# bias column order
IB_WI = 0
IB_IRES1 = 1               # 1..3
IB_IRES2 = 4               # 4..6
IB_WINT = 7
IB_ARES1 = 8               # 8..9
IB_ARES2 = 10              # 10..11
IB_UGATE = 12
IB_NLN2 = 13
IB_HALF = 14
NB = 15

_GRAPH = None


class _Bacc(bacc.Bacc):
    """Bacc with act-table preference reordered so the single table covering
    Exp+Ln+Copy (natural_log_exp_and_others) is picked for every activation,
    avoiding per-op table reload thrash."""

    def insert_act_table_loads(self):
        import concourse.mybir as _mb
        from concourse.hw_specs import get_activation_tables
        import bass_rust as _br

        has_activation = any(
            isinstance(i, _mb.InstActivation)
            for b in self.main_func.blocks
            for i in b.instructions
        )
        if not has_activation:
            return
        tables = [
            (name, s if name == "natural_log_exp_and_others" else set())
            for name, s in get_activation_tables(self.m.arch).items()
        ]
        _br.insert_act_table_loads(self, tables)


def build_graph():
    nc = _Bacc()

    x_in = nc.declare_dram_parameter("x", [BPC, N, F], F32, isOutput=False)
    rbfT_in = nc.declare_dram_parameter("rbfT", [K, BPC * E_B], BF16, isOutput=False)
    idx_in = nc.declare_dram_parameter("idx", [128, BPC * E_G // 16], I16, isOutput=False)
    h_in = nc.declare_dram_parameter("hmat", [BPC, 2, 128, E_H], FP8, isOutput=False)
    w_in = nc.declare_dram_parameter("wstack", [F, 12, F], F32R, isOutput=False)
    b_in = nc.declare_dram_parameter("bstack", [F, NB], F32, isOutput=False)
    k2fT_in = nc.declare_dram_parameter("k2fT", [K, F], BF16, isOutput=False)
    sel_in = nc.declare_dram_parameter("selbuf", [128, 66], BF16, isOutput=False)
    id_in = nc.declare_dram_parameter("ident", [128, 128], F32, isOutput=False)
    out_ext = nc.declare_dram_parameter("out", [BPC, N, F], F32, isOutput=True)

    with tile.TileContext(nc) as tc, ExitStack() as ctx:
        const = ctx.enter_context(tc.tile_pool(name="const", bufs=1))
        pro = ctx.enter_context(tc.tile_pool(name="pro", bufs=1))
        pairp = ctx.enter_context(tc.tile_pool(name="pairp", bufs=1))
        small = ctx.enter_context(tc.tile_pool(name="small", bufs=2))
        rbfp = ctx.enter_context(tc.tile_pool(name="rbfp", bufs=2))
        hp = ctx.enter_context(tc.tile_pool(name="hp", bufs=2))
        xjp = ctx.enter_context(tc.tile_pool(name="xjp", bufs=3))
        up = ctx.enter_context(tc.tile_pool(name="up", bufs=8))
        mlp = ctx.enter_context(tc.tile_pool(name="mlp", bufs=1))
        mlph = ctx.enter_context(tc.tile_pool(name="mlph", bufs=4))
        dramp = ctx.enter_context(tc.tile_pool(name="dramp", bufs=4, space="DRAM"))
        # PSUM budget: g(2 bufs)=2 banks, xh(2)=2, sT parity(1+1)=2,
        # z parity(1+1)=2 -> exactly 8 banks. Batch-parity tags decouple
        # adjacent batches' serial MLP chains (static round-robin buffer
        # assignment would otherwise serialize them).
        psg = ctx.enter_context(tc.tile_pool(name="psg", bufs=2, space="PSUM"))
        psh = ctx.enter_context(tc.tile_pool(name="psh", bufs=2, space="PSUM"))
        pss = ctx.enter_context(tc.tile_pool(name="pss", bufs=1, space="PSUM"))
        pz = ctx.enter_context(tc.tile_pool(name="pz", bufs=1, space="PSUM"))

        # constants (ordered so the gather-critical ones land first;
        # w_sb is only needed by the MLP and loads after batch 0's stream)
        idx_sb = const.tile([128, BPC * E_G // 16], I16)
        nc.sync.dma_start(out=idx_sb[:, :], in_=idx_in[:, :])
        b_sb = const.tile([F, NB], F32)
        nc.sync.dma_start(out=b_sb[:], in_=b_in[:, :])
        k2fT_sb = const.tile([K, F], BF16)
        nc.sync.dma_start(out=k2fT_sb[:], in_=k2fT_in[:, :])
        sel_sb = const.tile([128, 66], BF16)
        nc.sync.dma_start(out=sel_sb[:], in_=sel_in[:, :])
        ident = const.tile([128, 128], F32)
        nc.sync.dma_start(out=ident[:], in_=id_in[:, :])
        w_sb = const.tile([F, 12, F], F32R)

        def bias(col):
            return b_sb[:, col : col + 1]

        def softplus(dst, src, pre_bias, tmp_pool, tmp_tag):
            e = tmp_pool.tile(list(dst.shape), F32, tag=tmp_tag, name=f"e_{tmp_tag}")
            if pre_bias is None:
                nc.scalar.activation(e[:], src, AF.Exp)
            else:
                nc.scalar.activation(e[:], src, AF.Exp, bias=pre_bias)
            nc.scalar.activation(dst, e[:], AF.Ln, bias=1.0)

        def mm(out, lhsT, rhs, **kw):
            nc.tensor.matmul(out, lhsT, rhs, **kw)

        # ---- prologue: x load + softplus + gather-source write for ALL
        # batches up front, so the SP DMA queue is never head-of-line
        # blocked by a compute-dependent write mid-stream. The xa/out
        # writes go out on the Activation engine's HWDGE queue.
        xrows, xashs, xa_drams = [], [], []
        for b in range(BPC):
            xrow = pro.tile([128, 2, F], F32, tag=f"xrow{b}", name=f"xrow{b}")
            nc.sync.dma_start(
                out=xrow[:], in_=x_in[b].rearrange("(t p) f -> p t f", p=128)
            )
            # xash = softplus(x) - ln2 = Ln(exp(x)*0.5 + 0.5), bf16
            spe = small.tile([128, 2, F], F32, tag="spe")
            nc.scalar.activation(spe[:], xrow[:], AF.Exp)
            xash = pro.tile([128, 2, F], BF16, tag=f"xash{b}", name=f"xash{b}")
            nc.scalar.activation(
                xash[:], spe[:], AF.Ln, bias=bias(IB_HALF), scale=bias(IB_HALF)
            )
            xa_dram = dramp.tile([N, F], BF16, tag="xad")
            nc.scalar.dma_start(
                out=xa_dram[:].rearrange("(t p) f -> p t f", p=128), in_=xash[:]
            )
            xrows.append(xrow)
            xashs.append(xash)
            xa_drams.append(xa_dram)

        # MLP weights: not needed until the first batch's MLP, so load
        # after the prologue x traffic
        nc.sync.dma_start(out=w_sb[:], in_=w_in[:, :, :])

        # Per-batch MLP chains, emitted as GENERATORS advanced between the
        # next batch's aggregation groups. This interleaves the (serial,
        # latency-bound) chain ops with aggregation work in program order,
        # so the in-order engine queues never head-of-line block one stream
        # behind the other. Only the last batch's chain runs bare (the tail).
        def mlp_chain(b, P, xT, xi_sp, s_ps):
            def zt():
                return pz.tile([128, N], F32, tag=f"z{P}", name=f"z{P}")

            def sp(dst, src, pre_bias):
                e = mlp.tile(list(dst.shape), F32, tag=f"et{P}", name=f"e{P}")
                if pre_bias is None:
                    nc.scalar.activation(e[:], src, AF.Exp)
                else:
                    nc.scalar.activation(e[:], src, AF.Exp, bias=pre_bias)
                yield
                nc.scalar.activation(dst, e[:], AF.Ln, bias=1.0)
                yield

            # ---- assemble h = xi + sT ------------------------------------
            s_sb = small.tile([128, 2, 128], F32, tag="ssb")
            sT2 = zt()
            for t in range(2):
                nc.scalar.activation(s_sb[:, t, :], s_ps[:, t, :], AF.Copy)
                yield
                nc.tensor.transpose(
                    sT2[:, t * 128 : (t + 1) * 128], s_sb[:, t, :], ident[:]
                )
                yield
            h = mlph.tile([128, N], F32, tag=f"h{P}")
            nc.vector.tensor_add(h[:], xi_sp[:], sT2[:])
            yield
            # ---- interaction res blocks (carrier h = v_true + ln2) -------
            for l in range(3):
                a1 = mlp.tile([128, N], F32R, tag=f"a1{P}")
                yield from sp(a1[:], h[:], bias(IB_NLN2))
                z1 = zt()
                mm(z1[:], w_sb[:, IW_IRES + 2 * l, :], a1[:], start=True, stop=True)
                yield
                a2 = mlp.tile([128, N], F32R, tag=f"a2{P}")
                yield from sp(a2[:], z1[:], bias(IB_IRES1 + l))
                z2 = zt()
                mm(z2[:], w_sb[:, IW_IRES + 2 * l + 1, :], a2[:], start=True, stop=True)
                yield
                h2 = mlph.tile([128, N], F32, tag=f"h{P}")
                nc.vector.scalar_tensor_tensor(
                    h2[:], z2[:], bias(IB_IRES2 + l), h[:], ALU.add, ALU.add
                )
                yield
                h = h2
            # ---- gate: out0 = u_gate*x + v @ Wint.T + bint_adj -----------
            av = mlp.tile([128, N], F32R, tag=f"a1{P}")
            yield from sp(av[:], h[:], bias(IB_NLN2))
            zv = zt()
            mm(zv[:], w_sb[:, IW_WINT, :], av[:], start=True, stop=True)
            yield
            gx = mlp.tile([128, N], F32, tag=f"a2{P}")
            nc.gpsimd.tensor_scalar_mul(
                gx[:], xT[:].rearrange("p t f -> p (t f)"), bias(IB_UGATE)
            )
            yield
            h = mlph.tile([128, N], F32, tag=f"h{P}")
            nc.vector.scalar_tensor_tensor(
                h[:], zv[:], bias(IB_WINT), gx[:], ALU.add, ALU.add
            )
            yield
            # ---- atom res blocks (true-valued carrier) -------------------
            for l in range(2):
                a1 = mlp.tile([128, N], F32R, tag=f"a1{P}")
                yield from sp(a1[:], h[:], None)
                z1 = zt()
                mm(z1[:], w_sb[:, IW_ARES + 2 * l, :], a1[:], start=True, stop=True)
                yield
                a2 = mlp.tile([128, N], F32R, tag=f"a2{P}")
                yield from sp(a2[:], z1[:], bias(IB_ARES1 + l))
                z2 = zt()
                mm(z2[:], w_sb[:, IW_ARES + 2 * l + 1, :], a2[:], start=True, stop=True)
                yield
                h2 = mlph.tile([128, N], F32, tag=f"h{P}")
                nc.vector.scalar_tensor_tensor(
                    h2[:], z2[:], bias(IB_ARES2 + l), h[:], ALU.add, ALU.add
                )
                yield
                h = h2
            # ---- output: transpose back to row-major ---------------------
            oT_ps = zt()
            o_sb = small.tile([128, 2, 128], F32, tag="osb")
            for t in range(2):
                nc.tensor.transpose(
                    oT_ps[:, t * 128 : (t + 1) * 128],
                    h[:, t * 128 : (t + 1) * 128],
                    ident[:],
                )
                yield
                nc.scalar.activation(
                    o_sb[:, t, :], oT_ps[:, t * 128 : (t + 1) * 128], AF.Copy
                )
                yield
                nc.scalar.dma_start(
                    out=out_ext[b, t * 128 : (t + 1) * 128, :], in_=o_sb[:, t, :]
                )
                yield

        pending = None
        for b in range(BPC):
            xrow, xash, xa_dram = xrows[b], xashs[b], xa_drams[b]
            P = b % 2

            # feature-major xT (for gate term) and sp(xT) (for xi matmul)
            xT = small.tile([128, 2, 128], F32, tag="xT")
            xaT = small.tile([128, 2, 128], F32R, tag="xaT")
            xT_ps = pz.tile([128, N], F32, tag=f"z{P}", name=f"z{P}")
            for t in range(2):
                nc.tensor.transpose(
                    xT_ps[:, t * 128 : (t + 1) * 128], xrow[:, t, :], ident[:]
                )
                nc.scalar.activation(
                    xT[:, t, :], xT_ps[:, t * 128 : (t + 1) * 128], AF.Copy
                )
                softplus(
                    xaT[:, t, :], xT_ps[:, t * 128 : (t + 1) * 128], None,
                    small, "xaTe",
                )

            # xi = softplus(zi + bi_adj); carrier v+ln2 = xi_sp + sT
            zi_ps = pz.tile([128, N], F32, tag=f"z{P}", name=f"z{P}")
            mm(
                zi_ps[:],
                w_sb[:, IW_WI, :],
                xaT[:].rearrange("p t f -> p (t f)"),
                start=True,
                stop=True,
            )
            xi_sp = mlp.tile([128, N], F32, tag=f"xi{P}")
            softplus(xi_sp[:], zi_ps[:], bias(IB_WI), mlp, f"et{P}")

            # ---- gather + one-hot H + g + u + reduce --------------------
            xj = xjp.tile([128, T_G, F], BF16, tag="xj")
            off = 0
            for c, gsz in enumerate(G_SIZES):
                col0 = b * (E_G // 16) + off // 16
                nc.gpsimd.dma_gather(
                    out_ap=xj[:, off // 128 : (off + gsz) // 128, :],
                    in_ap=xa_dram[:, :],
                    idxs_ap=idx_sb[:, col0 : col0 + gsz // 16],
                    num_idxs=gsz,
                    num_idxs_reg=gsz,
                    elem_size=F,
                    single_packet=False,
                )
                off += gsz

            # H-sourced groups are processed FIRST (their inputs need no
            # gather round-trip), so load H + the back rbf quarters first;
            # the gather lands under the H-group compute.
            rbfT_sb = rbfp.tile([K, E_B], BF16, tag="rbfT")
            h_sb = hp.tile([128, 2, E_H], FP8, tag="hmat")
            EQ = E_B // 4

            def load_rbf_q(q):
                nc.sync.dma_start(
                    out=rbfT_sb[:, q * EQ : (q + 1) * EQ],
                    in_=rbfT_in[:, b * E_B + q * EQ : b * E_B + (q + 1) * EQ],
                )

            def load_h_half(hh):
                nc.sync.dma_start(
                    out=h_sb[:, :, hh * (E_H // 2) : (hh + 1) * (E_H // 2)],
                    in_=h_in[
                        b, :, :, hh * (E_H // 2) : (hh + 1) * (E_H // 2)
                    ].rearrange("h p e -> p h e"),
                )

            if b == 0:
                # batch 0: H inputs first so the PE/DVE pipeline starts
                # ~15us earlier (H-groups need no gather round-trip)
                load_rbf_q(2)
                load_h_half(0)
                load_rbf_q(3)
                load_h_half(1)
            else:
                load_rbf_q(0)
                load_rbf_q(1)
                load_h_half(0)

            s_ps = pss.tile([128, 2, 128], F32, tag=f"sT{P}", name=f"sT{P}")
            if b == 0:
                # H-groups 20..31 first (whole 16-tile accumulation blocks
                # only -- one pending PSUM group at a time), then 0..19
                grp_order = list(range(GRP_SPLIT + 6, ET_B // 4)) + list(
                    range(GRP_SPLIT + 6)
                )
                deferred_loads = {2: lambda: load_rbf_q(0),
                                  6: lambda: load_rbf_q(1)}
            else:
                grp_order = list(range(ET_B // 4))
                deferred_loads = {3: lambda: load_rbf_q(2),
                                  6: lambda: (load_rbf_q(3), load_h_half(1))}
            tile_pos = {}
            for _i, _g in enumerate(grp_order):
                for _j in range(4):
                    tile_pos[_g * 4 + _j] = _i * 4 + _j
            for gi, grp in enumerate(grp_order):  # 32 groups of 4 tiles
                if gi in deferred_loads:
                    deferred_loads[gi]()
                g_ps = psg.tile([128, 4, 128], F32, tag="g")
                for j in range(4):
                    t = grp * 4 + j
                    nc.tensor.matmul(
                        g_ps[:, j, :],
                        rbfT_sb[:, t * 128 : (t + 1) * 128],
                        k2fT_sb[:],
                        start=True,
                        stop=True,
                    )
                u = up.tile([128, 4, 128], BF16, tag="u")
                if grp < GRP_SPLIT:
                    if b < 1:
                        # early batches: Act is idle here, so evacuate g to
                        # bf16 SBUF and run the multiply in DVE 2x mode
                        # (all-2-byte operands)
                        g_sb = up.tile([128, 4, 128], BF16, tag="gsb")
                        nc.scalar.activation(
                            g_sb[:].rearrange("p a f -> p (a f)"),
                            g_ps[:].rearrange("p a f -> p (a f)"),
                            AF.Copy,
                        )
                        nc.vector.tensor_mul(
                            u[:].rearrange("p a f -> p (a f)"),
                            g_sb[:].rearrange("p a f -> p (a f)"),
                            xj[:, grp * 4 : (grp + 1) * 4, :].rearrange(
                                "p a f -> p (a f)"
                            ),
                        )
                    else:
                        nc.vector.tensor_mul(
                            u[:].rearrange("p a f -> p (a f)"),
                            g_ps[:].rearrange("p a f -> p (a f)"),
                            xj[:, grp * 4 : (grp + 1) * 4, :].rearrange(
                                "p a f -> p (a f)"
                            ),
                        )
                else:
                    # one-hot-sourced tiles: xj = H_tile.T @ xash in PSUM.
                    # DVE can read only one PSUM operand, so evacuate xh to
                    # SBUF (bf16) on the Act engine before the multiply.
                    xh_ps = psh.tile([128, 4, 128], F32, tag="xh")
                    for j in range(4):
                        e0 = (grp - GRP_SPLIT) * 4 * 128 + j * 128
                        nc.tensor.matmul(
                            xh_ps[:, j, :],
                            h_sb[:, 0, e0 : e0 + 128],
                            xash[:, 0, :],
                            start=True,
                            stop=False,
                        )
                        nc.tensor.matmul(
                            xh_ps[:, j, :],
                            h_sb[:, 1, e0 : e0 + 128],
                            xash[:, 1, :],
                            start=False,
                            stop=True,
                        )
                    xh_sb = up.tile([128, 4, 128], BF16, tag="xhs")
                    nc.scalar.activation(
                        xh_sb[:].rearrange("p a f -> p (a f)"),
                        xh_ps[:].rearrange("p a f -> p (a f)"),
                        AF.Copy,
                    )
                    if b < 1:
                        g_sb = up.tile([128, 4, 128], BF16, tag="gsb")
                        nc.scalar.activation(
                            g_sb[:].rearrange("p a f -> p (a f)"),
                            g_ps[:].rearrange("p a f -> p (a f)"),
                            AF.Copy,
                        )
                        nc.vector.tensor_mul(
                            u[:].rearrange("p a f -> p (a f)"),
                            g_sb[:].rearrange("p a f -> p (a f)"),
                            xh_sb[:].rearrange("p a f -> p (a f)"),
                        )
                    else:
                        nc.vector.tensor_mul(
                            u[:].rearrange("p a f -> p (a f)"),
                            g_ps[:].rearrange("p a f -> p (a f)"),
                            xh_sb[:].rearrange("p a f -> p (a f)"),
                        )
                for j in range(4):
                    t = grp * 4 + j
                    # edge-tile t -> n-pair (2t, 2t+1); s-tile half = t // 64;
                    # 32-partition block bblk = (t % 64) // 16; slot r = t % 16.
                    # start/stop follow PROCESSING order (H-groups run first,
                    # so a block straddling the gather/H boundary starts at
                    # its first-processed tile).
                    half = t // 64
                    bblk = (t % 64) // 16
                    r = t % 16
                    blk0 = (t // 16) * 16
                    pos = [tile_pos[blk0 + rr] for rr in range(16)]
                    nc.tensor.matmul(
                        s_ps[32 * bblk : 32 * bblk + 32, half, :],
                        sel_sb[:, 32 - 2 * r : 64 - 2 * r],
                        u[:, j, :],
                        start=(tile_pos[t] == min(pos)),
                        stop=(tile_pos[t] == max(pos)),
                        tile_position=(0, 32 * bblk),
                    )
                # advance the previous batch's MLP chain a couple of ops
                if pending is not None:
                    for _ in range(2):
                        if next(pending, "done") == "done":
                            pending = None
                            break

            if pending is not None:
                for _ in pending:
                    pass
            pending = mlp_chain(b, P, xT, xi_sp, s_ps)

        if pending is not None:
            for _ in pending:
                pass

    nc.compile()
    return nc


def _prep_core_inputs(inputs):
    """Host-side layout prep. Returns in_maps for the 8 cores."""
    x = np.asarray(inputs["x"], np.float32)
    rbf = np.asarray(inputs["rbf"], np.float32)
    neighbor = np.asarray(inputs["neighbor"])
    k2f_W = np.asarray(inputs["k2f_W"], np.float32)

    c = LN2

    def lhsT(w):
        return np.ascontiguousarray(np.asarray(w, np.float32).T)

    # weight stack [F, 12, F]
    ws = np.zeros((F, 12, F), np.float32)
    ws[:, IW_WI, :] = lhsT(inputs["Wi"])
    for l in range(3):
        ws[:, IW_IRES + 2 * l, :] = lhsT(inputs["ires_W1"][l])
        ws[:, IW_IRES + 2 * l + 1, :] = lhsT(inputs["ires_W2"][l])
    ws[:, IW_WINT, :] = lhsT(inputs["Wint"])
    for l in range(2):
        ws[:, IW_ARES + 2 * l, :] = lhsT(inputs["ares_W1"][l])
        ws[:, IW_ARES + 2 * l + 1, :] = lhsT(inputs["ares_W2"][l])

    # bias stack [F, NB] (softplus shift folded in)
    bs = np.zeros((F, NB), np.float32)
    rs = lambda w: np.asarray(w, np.float32).sum(axis=1)
    bs[:, IB_NLN2] = -c
    bs[:, IB_HALF] = 0.5
    bs[:, IB_UGATE] = np.asarray(inputs["u_gate"], np.float32)
    bs[:, IB_WI] = inputs["bi"] - c * rs(inputs["Wi"])
    for l in range(3):
        bs[:, IB_IRES1 + l] = inputs["ires_b1"][l] - c * rs(inputs["ires_W1"][l])
        bs[:, IB_IRES2 + l] = inputs["ires_b2"][l] - c * rs(inputs["ires_W2"][l])
    bs[:, IB_WINT] = inputs["bint"] - c * rs(inputs["Wint"])
    for l in range(2):
        bs[:, IB_ARES1 + l] = inputs["ares_b1"][l] - c * rs(inputs["ares_W1"][l])
        bs[:, IB_ARES2 + l] = inputs["ares_b2"][l] - c * rs(inputs["ares_W2"][l])

    k2fT = np.ascontiguousarray(k2f_W.T).astype(ml_dtypes.bfloat16)  # [K, F]

    selbuf = np.zeros((128, 66), ml_dtypes.bfloat16)
    selbuf[:64, 32] = 1
    selbuf[64:, 33] = 1

    ident = np.eye(128, dtype=np.float32)

    in_maps = []
    eye256 = np.eye(256, dtype=ml_dtypes.float8_e4m3fn)
    for i in range(NCORES):
        bs_lo = i * BPC
        x_c = np.ascontiguousarray(x[bs_lo : bs_lo + BPC])
        rbf_c = rbf[bs_lo : bs_lo + BPC].reshape(BPC * E_B, K)
        rbfT_c = np.ascontiguousarray(rbf_c.T).astype(ml_dtypes.bfloat16)
        nbr_c = neighbor[bs_lo : bs_lo + BPC].reshape(BPC, E_B).astype(np.int64)
        # gather idxs: first E_G edges of each batch.
        # dma_gather wrap: idx i -> partition i%16, col i//16, per call
        idx_c = np.zeros((128, BPC * E_G // 16), np.int16)
        # one-hot H for the last E_H edges: [BPC, 2, 128, E_H]
        h_c = np.zeros((BPC, 2, 128, E_H), ml_dtypes.float8_e4m3fn)
        for b in range(BPC):
            nb_b = nbr_c[b]
            ng = nb_b[:E_G].astype(np.int16)
            off = 0
            for gsz in G_SIZES:
                seg = ng[off : off + gsz]
                wrap = np.tile(seg.reshape(gsz // 16, 16).T, (8, 1))
                col0 = b * (E_G // 16) + off // 16
                idx_c[:, col0 : col0 + gsz // 16] = wrap
                off += gsz
            nh = nb_b[E_G:]
            h_b = eye256[:, nh]  # [256, E_H] one-hot columns
            h_c[b, 0] = h_b[:128]
            h_c[b, 1] = h_b[128:]
        in_maps.append(
            {
                "x": x_c,
                "rbfT": rbfT_c,
                "idx": idx_c,
                "hmat": h_c,
                "wstack": ws,
                "bstack": bs,
                "k2fT": k2fT,
                "selbuf": selbuf,
                "ident": ident,
            }
        )
    return in_maps


def run(inputs, trace=False, **kwargs):
    global _GRAPH
    if _GRAPH is None:
        _GRAPH = build_graph()
    in_maps = _prep_core_inputs(inputs)
    res = run_bass_kernel_spmd(
        _GRAPH, in_maps, core_ids=list(range(NCORES)), trace=trace, **kwargs
    )
    outs = [np.asarray(res.results[i]["out"], np.float32) for i in range(NCORES)]
    full = np.concatenate(outs, axis=0)  # [B, N, F]
    return full, res


def kernel(**inputs):
    full, _ = run(inputs, trace=False)
    return full


# revision 79
# speedup vs baseline: 1.0120x; 1.0120x over previous
"""Trainium2 Bass kernel for AtomInteractionWithResidual (PhysNet-style GNN block).

Strategy (8 NeuronCores, data-parallel over batch B=32 -> 4 batches/core):
  - Host-side prep (layout only): rbf transposed to [K, edges] (bf16),
    MLP weights pre-transposed to lhsT form, softplus-shift (ln 2) folded
    into biases.
  - Neighbor-feature materialization is SPLIT per batch to balance the DMA
    engines against the PE array:
      * edges 0..9216   (T_G=72 tiles): dma_gather of softplus(x) rows from
        DRAM (bf16, 2 gather calls per batch)
      * edges 9216..16384 (T_H=56 tiles): one-hot matmul on the PE --
        xj_tile = H_tile.T @ xash where H is a host-built fp8 one-hot
        (exact 0/1 selection, 2 accumulating passes over the 256-atom
        contraction). H streams in as contiguous fp8 DMA, which is ~2x
        cheaper per edge than per-row gather descriptors. The resulting
        PSUM xj is evacuated to SBUF bf16 on the Act engine (DVE can only
        read one PSUM operand).
  - Device per batch:
      xa = softplus(x) - ln2 (bf16) -> DRAM (gather source) + SBUF (H rhs)
      g  = rbfT.T @ k2fT per 128-edge tile (PE)        [edge, F] f32 PSUM
      u  = g * xj (DVE)                                [edge, F] bf16
      sT += sel-window.T @ u (PE, accumulating)        [F, n]    feature-major
      feature-major MLP chain (interaction res blocks, gate, atom res blocks)
      using float32r matmuls (4x faster than f32 on the PE at 256-wide).
  - Scheduling structure (the cost model serializes all DMA on one device
    and every engine queue is in-order):
      * x->xa prologue for all batches runs first; compute-dependent writes
        (xa, out) issue on the Activation engine's HWDGE queue so they never
        head-of-line block the SP bulk-load queue.
      * PSUM pools use batch-parity tags (sT0/sT1, z0/z1) so adjacent
        batches' serial MLP chains get disjoint banks and can overlap.
      * Each batch's MLP chain is emitted as a generator advanced between
        the next batch's aggregation groups (program-order interleaving).
      * Batch 0 processes one-hot groups first (their inputs need no
        DRAM round-trip), starting the PE/DVE pipeline ~15us earlier.
"""

import numpy as np
import ml_dtypes
from contextlib import ExitStack

import concourse.bass as bass
from concourse import bacc
import concourse.mybir as mybir
import concourse.tile as tile
from concourse.bass_utils import run_bass_kernel_spmd

F32 = mybir.dt.float32
F32R = mybir.dt.float32r
BF16 = mybir.dt.bfloat16
FP8 = mybir.dt.float8e4
I16 = mybir.dt.int16
AF = mybir.ActivationFunctionType
ALU = mybir.AluOpType

B, N, M, F, K = 32, 256, 64, 128, 64
NCORES = 8
BPC = B // NCORES          # batches per core
E_B = N * M                # edges per batch (16384)
ET_B = E_B // 128          # 128-edge tiles per batch (128)
T_G = 56                   # tiles materialized via dma_gather
T_H = ET_B - T_G           # tiles materialized via one-hot matmul (72)
E_G = T_G * 128            # gathered edges per batch (7168)
E_H = T_H * 128            # one-hot edges per batch (9216)
G_SIZES = [E_G // 2, E_G // 2]  # gather call sizes
assert sum(G_SIZES) == E_G
GRP_SPLIT = T_G // 4       # group index where the H-sourced tiles start (14)
LN2 = float(np.log(2.0))

# weight stack order (lhsT = W.T each)
IW_WI = 0
IW_IRES = 1                # 1..6: (W1,W2) x 3
IW_WINT = 7
IW_ARES = 8                # 8..11: (W1,W2) x 2
# bias column order
IB_WI = 0
IB_IRES1 = 1               # 1..3
IB_IRES2 = 4               # 4..6
IB_WINT = 7
IB_ARES1 = 8               # 8..9
IB_ARES2 = 10              # 10..11
IB_UGATE = 12
IB_NLN2 = 13
IB_HALF = 14
NB = 15

_GRAPH = None


class _Bacc(bacc.Bacc):
    """Bacc with act-table preference reordered so the single table covering
    Exp+Ln+Copy (natural_log_exp_and_others) is picked for every activation,
    avoiding per-op table reload thrash."""

    def insert_act_table_loads(self):
        import concourse.mybir as _mb
        from concourse.hw_specs import get_activation_tables
        import bass_rust as _br

        has_activation = any(
            isinstance(i, _mb.InstActivation)
            for b in self.main_func.blocks
            for i in b.instructions
        )
        if not has_activation:
            return
        tables = [
            (name, s if name == "natural_log_exp_and_others" else set())
            for name, s in get_activation_tables(self.m.arch).items()
        ]
        _br.insert_act_table_loads(self, tables)


def build_graph():
    nc = _Bacc()

    x_in = nc.declare_dram_parameter("x", [BPC, N, F], F32, isOutput=False)
    rbfT_in = nc.declare_dram_parameter("rbfT", [K, BPC * E_B], BF16, isOutput=False)
    idx_in = nc.declare_dram_parameter("idx", [128, BPC * E_G // 16], I16, isOutput=False)
    h_in = nc.declare_dram_parameter("hmat", [BPC, 2, 128, E_H], FP8, isOutput=False)
    w_in = nc.declare_dram_parameter("wstack", [F, 12, F], F32R, isOutput=False)
    b_in = nc.declare_dram_parameter("bstack", [F, NB], F32, isOutput=False)
    k2fT_in = nc.declare_dram_parameter("k2fT", [K, F], BF16, isOutput=False)
    sel_in = nc.declare_dram_parameter("selbuf", [128, 66], BF16, isOutput=False)
    id_in = nc.declare_dram_parameter("ident", [128, 128], F32, isOutput=False)
    out_ext = nc.declare_dram_parameter("out", [BPC, N, F], F32, isOutput=True)

    with tile.TileContext(nc) as tc, ExitStack() as ctx:
        const = ctx.enter_context(tc.tile_pool(name="const", bufs=1))
        pro = ctx.enter_context(tc.tile_pool(name="pro", bufs=1))
        pairp = ctx.enter_context(tc.tile_pool(name="pairp", bufs=1))
        small = ctx.enter_context(tc.tile_pool(name="small", bufs=2))
        rbfp = ctx.enter_context(tc.tile_pool(name="rbfp", bufs=2))
        hp = ctx.enter_context(tc.tile_pool(name="hp", bufs=2))
        xjp = ctx.enter_context(tc.tile_pool(name="xjp", bufs=3))
        up = ctx.enter_context(tc.tile_pool(name="up", bufs=8))
        mlp = ctx.enter_context(tc.tile_pool(name="mlp", bufs=2))
        mlph = ctx.enter_context(tc.tile_pool(name="mlph", bufs=4))
        dramp = ctx.enter_context(tc.tile_pool(name="dramp", bufs=4, space="DRAM"))
        # PSUM budget: g(2 bufs)=2 banks, xh(2)=2, sT parity(1+1)=2,
        # z parity(1+1)=2 -> exactly 8 banks. Batch-parity tags decouple
        # adjacent batches' serial MLP chains (static round-robin buffer
        # assignment would otherwise serialize them).
        psg = ctx.enter_context(tc.tile_pool(name="psg", bufs=2, space="PSUM"))
        psh = ctx.enter_context(tc.tile_pool(name="psh", bufs=2, space="PSUM"))
        pss = ctx.enter_context(tc.tile_pool(name="pss", bufs=1, space="PSUM"))
        pz = ctx.enter_context(tc.tile_pool(name="pz", bufs=1, space="PSUM"))

        # constants (ordered so the gather-critical ones land first;
        # w_sb is only needed by the MLP and loads after batch 0's stream)
        idx_sb = const.tile([128, BPC * E_G // 16], I16)
        nc.sync.dma_start(out=idx_sb[:, :], in_=idx_in[:, :])
        b_sb = const.tile([F, NB], F32)
        nc.sync.dma_start(out=b_sb[:], in_=b_in[:, :])
        k2fT_sb = const.tile([K, F], BF16)
        nc.sync.dma_start(out=k2fT_sb[:], in_=k2fT_in[:, :])
        sel_sb = const.tile([128, 66], BF16)
        nc.sync.dma_start(out=sel_sb[:], in_=sel_in[:, :])
        ident = const.tile([128, 128], F32)
        nc.sync.dma_start(out=ident[:], in_=id_in[:, :])
        w_sb = const.tile([F, 12, F], F32R)

        def bias(col):
            return b_sb[:, col : col + 1]

        def softplus(dst, src, pre_bias, tmp_pool, tmp_tag):
            e = tmp_pool.tile(list(dst.shape), F32, tag=tmp_tag, name=f"e_{tmp_tag}")
            if pre_bias is None:
                nc.scalar.activation(e[:], src, AF.Exp)
            else:
                nc.scalar.activation(e[:], src, AF.Exp, bias=pre_bias)
            nc.scalar.activation(dst, e[:], AF.Ln, bias=1.0)

        def mm(out, lhsT, rhs, **kw):
            nc.tensor.matmul(out, lhsT, rhs, **kw)

        # ---- prologue: x load + softplus + gather-source write for ALL
        # batches up front, so the SP DMA queue is never head-of-line
        # blocked by a compute-dependent write mid-stream. The xa/out
        # writes go out on the Activation engine's HWDGE queue.
        xrows, xashs, xa_drams = [], [], []
        for b in range(BPC):
            xrow = pro.tile([128, 2, F], F32, tag=f"xrow{b}", name=f"xrow{b}")
            nc.sync.dma_start(
                out=xrow[:], in_=x_in[b].rearrange("(t p) f -> p t f", p=128)
            )
            # xash = softplus(x) - ln2 = Ln(exp(x)*0.5 + 0.5), bf16
            spe = small.tile([128, 2, F], F32, tag="spe")
            nc.scalar.activation(spe[:], xrow[:], AF.Exp)
            xash = pro.tile([128, 2, F], BF16, tag=f"xash{b}", name=f"xash{b}")
            nc.scalar.activation(
                xash[:], spe[:], AF.Ln, bias=bias(IB_HALF), scale=bias(IB_HALF)
            )
            xa_dram = dramp.tile([N, F], BF16, tag="xad")
            nc.scalar.dma_start(
                out=xa_dram[:].rearrange("(t p) f -> p t f", p=128), in_=xash[:]
            )
            xrows.append(xrow)
            xashs.append(xash)
            xa_drams.append(xa_dram)

        # MLP weights: not needed until the first batch's MLP, so load
        # after the prologue x traffic
        nc.sync.dma_start(out=w_sb[:], in_=w_in[:, :, :])

        # Per-batch MLP chains, emitted as GENERATORS advanced between the
        # next batch's aggregation groups. This interleaves the (serial,
        # latency-bound) chain ops with aggregation work in program order,
        # so the in-order engine queues never head-of-line block one stream
        # behind the other. Only the last batch's chain runs bare (the tail).
        def mlp_chain(b, P, xT, xi_sp, s_ps):
            def zt():
                return pz.tile([128, N], F32, tag=f"z{P}", name=f"z{P}")

            def sp(dst, src, pre_bias):
                e = mlp.tile(list(dst.shape), F32, tag=f"et{P}", name=f"e{P}")
                if pre_bias is None:
                    nc.scalar.activation(e[:], src, AF.Exp)
                else:
                    nc.scalar.activation(e[:], src, AF.Exp, bias=pre_bias)
                yield
                nc.scalar.activation(dst, e[:], AF.Ln, bias=1.0)
                yield

            # ---- assemble h = xi + sT ------------------------------------
            s_sb = small.tile([128, 2, 128], F32, tag="ssb")
            sT2 = zt()
            for t in range(2):
                nc.scalar.activation(s_sb[:, t, :], s_ps[:, t, :], AF.Copy)
                yield
                nc.tensor.transpose(
                    sT2[:, t * 128 : (t + 1) * 128], s_sb[:, t, :], ident[:]
                )
                yield
            h = mlph.tile([128, N], F32, tag=f"h{P}")
            nc.vector.tensor_add(h[:], xi_sp[:], sT2[:])
            yield
            # ---- interaction res blocks (carrier h = v_true + ln2) -------
            for l in range(3):
                a1 = mlp.tile([128, N], F32R, tag=f"a1{P}")
                yield from sp(a1[:], h[:], bias(IB_NLN2))
                z1 = zt()
                mm(z1[:], w_sb[:, IW_IRES + 2 * l, :], a1[:], start=True, stop=True)
                yield
                a2 = mlp.tile([128, N], F32R, tag=f"a2{P}")
                yield from sp(a2[:], z1[:], bias(IB_IRES1 + l))
                z2 = zt()
                mm(z2[:], w_sb[:, IW_IRES + 2 * l + 1, :], a2[:], start=True, stop=True)
                yield
                h2 = mlph.tile([128, N], F32, tag=f"h{P}")
                nc.vector.scalar_tensor_tensor(
                    h2[:], z2[:], bias(IB_IRES2 + l), h[:], ALU.add, ALU.add
                )
                yield
                h = h2
            # ---- gate: out0 = u_gate*x + v @ Wint.T + bint_adj -----------
            av = mlp.tile([128, N], F32R, tag=f"a1{P}")
            yield from sp(av[:], h[:], bias(IB_NLN2))
            zv = zt()
            mm(zv[:], w_sb[:, IW_WINT, :], av[:], start=True, stop=True)
            yield
            gx = mlp.tile([128, N], F32, tag=f"a2{P}")
            nc.gpsimd.tensor_scalar_mul(
                gx[:], xT[:].rearrange("p t f -> p (t f)"), bias(IB_UGATE)
            )
            yield
            h = mlph.tile([128, N], F32, tag=f"h{P}")
            nc.vector.scalar_tensor_tensor(
                h[:], zv[:], bias(IB_WINT), gx[:], ALU.add, ALU.add
            )
            yield
            # ---- atom res blocks (true-valued carrier) -------------------
            for l in range(2):
                a1 = mlp.tile([128, N], F32R, tag=f"a1{P}")
                yield from sp(a1[:], h[:], None)
                z1 = zt()
                mm(z1[:], w_sb[:, IW_ARES + 2 * l, :], a1[:], start=True, stop=True)
                yield
                a2 = mlp.tile([128, N], F32R, tag=f"a2{P}")
                yield from sp(a2[:], z1[:], bias(IB_ARES1 + l))
                z2 = zt()
                mm(z2[:], w_sb[:, IW_ARES + 2 * l + 1, :], a2[:], start=True, stop=True)
                yield
                h2 = mlph.tile([128, N], F32, tag=f"h{P}")
                nc.vector.scalar_tensor_tensor(
                    h2[:], z2[:], bias(IB_ARES2 + l), h[:], ALU.add, ALU.add
                )
                yield
                h = h2
            # ---- output: transpose back to row-major ---------------------
            oT_ps = zt()
            o_sb = small.tile([128, 2, 128], F32, tag="osb")
            for t in range(2):
                nc.tensor.transpose(
                    oT_ps[:, t * 128 : (t + 1) * 128],
                    h[:, t * 128 : (t + 1) * 128],
                    ident[:],
                )
                yield
                nc.scalar.activation(
                    o_sb[:, t, :], oT_ps[:, t * 128 : (t + 1) * 128], AF.Copy
                )
                yield
                nc.scalar.dma_start(
                    out=out_ext[b, t * 128 : (t + 1) * 128, :], in_=o_sb[:, t, :]
                )
                yield

        pending = None
        for b in range(BPC):
            xrow, xash, xa_dram = xrows[b], xashs[b], xa_drams[b]
            P = b % 2

            # feature-major xT (for gate term) and sp(xT) (for xi matmul)
            xT = small.tile([128, 2, 128], F32, tag="xT")
            xaT = small.tile([128, 2, 128], F32R, tag="xaT")
            xT_ps = pz.tile([128, N], F32, tag=f"z{P}", name=f"z{P}")
            for t in range(2):
                nc.tensor.transpose(
                    xT_ps[:, t * 128 : (t + 1) * 128], xrow[:, t, :], ident[:]
                )
                nc.scalar.activation(
                    xT[:, t, :], xT_ps[:, t * 128 : (t + 1) * 128], AF.Copy
                )
                softplus(
                    xaT[:, t, :], xT_ps[:, t * 128 : (t + 1) * 128], None,
                    small, "xaTe",
                )

            # xi = softplus(zi + bi_adj); carrier v+ln2 = xi_sp + sT
            zi_ps = pz.tile([128, N], F32, tag=f"z{P}", name=f"z{P}")
            mm(
                zi_ps[:],
                w_sb[:, IW_WI, :],
                xaT[:].rearrange("p t f -> p (t f)"),
                start=True,
                stop=True,
            )
            xi_sp = mlp.tile([128, N], F32, tag=f"xi{P}")
            softplus(xi_sp[:], zi_ps[:], bias(IB_WI), mlp, f"et{P}")

            # ---- gather + one-hot H + g + u + reduce --------------------
            xj = xjp.tile([128, T_G, F], BF16, tag="xj")
            off = 0
            for c, gsz in enumerate(G_SIZES):
                col0 = b * (E_G // 16) + off // 16
                nc.gpsimd.dma_gather(
                    out_ap=xj[:, off // 128 : (off + gsz) // 128, :],
                    in_ap=xa_dram[:, :],
                    idxs_ap=idx_sb[:, col0 : col0 + gsz // 16],
                    num_idxs=gsz,
                    num_idxs_reg=gsz,
                    elem_size=F,
                    single_packet=False,
                )
                off += gsz

            # H-sourced groups are processed FIRST (their inputs need no
            # gather round-trip), so load H + the back rbf quarters first;
            # the gather lands under the H-group compute.
            rbfT_sb = rbfp.tile([K, E_B], BF16, tag="rbfT")
            h_sb = hp.tile([128, 2, E_H], FP8, tag="hmat")
            EQ = E_B // 4

            def load_rbf_q(q):
                nc.sync.dma_start(
                    out=rbfT_sb[:, q * EQ : (q + 1) * EQ],
                    in_=rbfT_in[:, b * E_B + q * EQ : b * E_B + (q + 1) * EQ],
                )

            def load_h_half(hh):
                nc.sync.dma_start(
                    out=h_sb[:, :, hh * (E_H // 2) : (hh + 1) * (E_H // 2)],
                    in_=h_in[
                        b, :, :, hh * (E_H // 2) : (hh + 1) * (E_H // 2)
                    ].rearrange("h p e -> p h e"),
                )

            if b == 0:
                # batch 0: H inputs first so the PE/DVE pipeline starts
                # ~15us earlier (H-groups need no gather round-trip)
                load_rbf_q(2)
                load_h_half(0)
                load_rbf_q(3)
                load_h_half(1)
            else:
                load_rbf_q(0)
                load_rbf_q(1)
                load_h_half(0)

            s_ps = pss.tile([128, 2, 128], F32, tag=f"sT{P}", name=f"sT{P}")
            if b == 0:
                # H-groups 20..31 first (whole 16-tile accumulation blocks
                # only -- one pending PSUM group at a time), then 0..19
                grp_order = list(range(GRP_SPLIT + 6, ET_B // 4)) + list(
                    range(GRP_SPLIT + 6)
                )
                deferred_loads = {2: lambda: load_rbf_q(0),
                                  6: lambda: load_rbf_q(1)}
            else:
                grp_order = list(range(ET_B // 4))
                deferred_loads = {3: lambda: load_rbf_q(2),
                                  6: lambda: (load_rbf_q(3), load_h_half(1))}
            tile_pos = {}
            for _i, _g in enumerate(grp_order):
                for _j in range(4):
                    tile_pos[_g * 4 + _j] = _i * 4 + _j
            for gi, grp in enumerate(grp_order):  # 32 groups of 4 tiles
                if gi in deferred_loads:
                    deferred_loads[gi]()
                g_ps = psg.tile([128, 4, 128], F32, tag="g")
                for j in range(4):
                    t = grp * 4 + j
                    nc.tensor.matmul(
                        g_ps[:, j, :],
                        rbfT_sb[:, t * 128 : (t + 1) * 128],
                        k2fT_sb[:],
                        start=True,
                        stop=True,
                    )
                u = up.tile([128, 4, 128], BF16, tag="u")
                if grp < GRP_SPLIT:
                    if b < 1:
                        # early batches: Act is idle here, so evacuate g to
                        # bf16 SBUF and run the multiply in DVE 2x mode
                        # (all-2-byte operands)
                        g_sb = up.tile([128, 4, 128], BF16, tag="gsb")
                        nc.scalar.activation(
                            g_sb[:].rearrange("p a f -> p (a f)"),
                            g_ps[:].rearrange("p a f -> p (a f)"),
                            AF.Copy,
                        )
                        nc.vector.tensor_mul(
                            u[:].rearrange("p a f -> p (a f)"),
                            g_sb[:].rearrange("p a f -> p (a f)"),
                            xj[:, grp * 4 : (grp + 1) * 4, :].rearrange(
                                "p a f -> p (a f)"
                            ),
                        )
                    else:
                        nc.vector.tensor_mul(
                            u[:].rearrange("p a f -> p (a f)"),
                            g_ps[:].rearrange("p a f -> p (a f)"),
                            xj[:, grp * 4 : (grp + 1) * 4, :].rearrange(
                                "p a f -> p (a f)"
                            ),
                        )
                else:
                    # one-hot-sourced tiles: xj = H_tile.T @ xash in PSUM.
                    # DVE can read only one PSUM operand, so evacuate xh to
                    # SBUF (bf16) on the Act engine before the multiply.
                    xh_ps = psh.tile([128, 4, 128], F32, tag="xh")
                    for j in range(4):
                        e0 = (grp - GRP_SPLIT) * 4 * 128 + j * 128
                        nc.tensor.matmul(
                            xh_ps[:, j, :],
                            h_sb[:, 0, e0 : e0 + 128],
                            xash[:, 0, :],
                            start=True,
                            stop=False,
                        )
                        nc.tensor.matmul(
                            xh_ps[:, j, :],
                            h_sb[:, 1, e0 : e0 + 128],
                            xash[:, 1, :],
                            start=False,
                            stop=True,
                        )
                    xh_sb = up.tile([128, 4, 128], BF16, tag="xhs")
                    nc.scalar.activation(
                        xh_sb[:].rearrange("p a f -> p (a f)"),
                        xh_ps[:].rearrange("p a f -> p (a f)"),
                        AF.Copy,
                    )
                    if b < 1:
                        g_sb = up.tile([128, 4, 128], BF16, tag="gsb")
                        nc.scalar.activation(
                            g_sb[:].rearrange("p a f -> p (a f)"),
                            g_ps[:].rearrange("p a f -> p (a f)"),
                            AF.Copy,
                        )
                        nc.vector.tensor_mul(
                            u[:].rearrange("p a f -> p (a f)"),
                            g_sb[:].rearrange("p a f -> p (a f)"),
                            xh_sb[:].rearrange("p a f -> p (a f)"),
                        )
                    else:
                        nc.vector.tensor_mul(
                            u[:].rearrange("p a f -> p (a f)"),
                            g_ps[:].rearrange("p a f -> p (a f)"),
                            xh_sb[:].rearrange("p a f -> p (a f)"),
                        )
                for j in range(4):
                    t = grp * 4 + j
                    # edge-tile t -> n-pair (2t, 2t+1); s-tile half = t // 64;
                    # 32-partition block bblk = (t % 64) // 16; slot r = t % 16.
                    # start/stop follow PROCESSING order (H-groups run first,
                    # so a block straddling the gather/H boundary starts at
                    # its first-processed tile).
                    half = t // 64
                    bblk = (t % 64) // 16
                    r = t % 16
                    blk0 = (t // 16) * 16
                    pos = [tile_pos[blk0 + rr] for rr in range(16)]
                    nc.tensor.matmul(
                        s_ps[32 * bblk : 32 * bblk + 32, half, :],
                        sel_sb[:, 32 - 2 * r : 64 - 2 * r],
                        u[:, j, :],
                        start=(tile_pos[t] == min(pos)),
                        stop=(tile_pos[t] == max(pos)),
                        tile_position=(0, 32 * bblk),
                    )
                # advance the previous batch's MLP chain a couple of ops
                if pending is not None:
                    for _ in range(2):
                        if next(pending, "done") == "done":
                            pending = None
                            break

            if pending is not None:
                for _ in pending:
                    pass
            pending = mlp_chain(b, P, xT, xi_sp, s_ps)

        if pending is not None:
            for _ in pending:
                pass

    nc.compile()
    return nc


def _prep_core_inputs(inputs):
    """Host-side layout prep. Returns in_maps for the 8 cores."""
    x = np.asarray(inputs["x"], np.float32)
    rbf = np.asarray(inputs["rbf"], np.float32)
    neighbor = np.asarray(inputs["neighbor"])
    k2f_W = np.asarray(inputs["k2f_W"], np.float32)

    c = LN2

    def lhsT(w):
        return np.ascontiguousarray(np.asarray(w, np.float32).T)

    # weight stack [F, 12, F]
    ws = np.zeros((F, 12, F), np.float32)
    ws[:, IW_WI, :] = lhsT(inputs["Wi"])
    for l in range(3):
        ws[:, IW_IRES + 2 * l, :] = lhsT(inputs["ires_W1"][l])
        ws[:, IW_IRES + 2 * l + 1, :] = lhsT(inputs["ires_W2"][l])
    ws[:, IW_WINT, :] = lhsT(inputs["Wint"])
    for l in range(2):
        ws[:, IW_ARES + 2 * l, :] = lhsT(inputs["ares_W1"][l])
        ws[:, IW_ARES + 2 * l + 1, :] = lhsT(inputs["ares_W2"][l])

    # bias stack [F, NB] (softplus shift folded in)
    bs = np.zeros((F, NB), np.float32)
    rs = lambda w: np.asarray(w, np.float32).sum(axis=1)
    bs[:, IB_NLN2] = -c
    bs[:, IB_HALF] = 0.5
    bs[:, IB_UGATE] = np.asarray(inputs["u_gate"], np.float32)
    bs[:, IB_WI] = inputs["bi"] - c * rs(inputs["Wi"])
    for l in range(3):
        bs[:, IB_IRES1 + l] = inputs["ires_b1"][l] - c * rs(inputs["ires_W1"][l])
        bs[:, IB_IRES2 + l] = inputs["ires_b2"][l] - c * rs(inputs["ires_W2"][l])
    bs[:, IB_WINT] = inputs["bint"] - c * rs(inputs["Wint"])
    for l in range(2):
        bs[:, IB_ARES1 + l] = inputs["ares_b1"][l] - c * rs(inputs["ares_W1"][l])
        bs[:, IB_ARES2 + l] = inputs["ares_b2"][l] - c * rs(inputs["ares_W2"][l])

    k2fT = np.ascontiguousarray(k2f_W.T).astype(ml_dtypes.bfloat16)  # [K, F]

    selbuf = np.zeros((128, 66), ml_dtypes.bfloat16)
    selbuf[:64, 32] = 1
    selbuf[64:, 33] = 1

    ident = np.eye(128, dtype=np.float32)

    in_maps = []
    eye256 = np.eye(256, dtype=ml_dtypes.float8_e4m3fn)
    for i in range(NCORES):
        bs_lo = i * BPC
        x_c = np.ascontiguousarray(x[bs_lo : bs_lo + BPC])
        rbf_c = rbf[bs_lo : bs_lo + BPC].reshape(BPC * E_B, K)
        rbfT_c = np.ascontiguousarray(rbf_c.T).astype(ml_dtypes.bfloat16)
        nbr_c = neighbor[bs_lo : bs_lo + BPC].reshape(BPC, E_B).astype(np.int64)
        # gather idxs: first E_G edges of each batch.
        # dma_gather wrap: idx i -> partition i%16, col i//16, per call
        idx_c = np.zeros((128, BPC * E_G // 16), np.int16)
        # one-hot H for the last E_H edges: [BPC, 2, 128, E_H]
        h_c = np.zeros((BPC, 2, 128, E_H), ml_dtypes.float8_e4m3fn)
        for b in range(BPC):
            nb_b = nbr_c[b]
            ng = nb_b[:E_G].astype(np.int16)
            off = 0
            for gsz in G_SIZES:
                seg = ng[off : off + gsz]
                wrap = np.tile(seg.reshape(gsz // 16, 16).T, (8, 1))
                col0 = b * (E_G // 16) + off // 16
                idx_c[:, col0 : col0 + gsz // 16] = wrap
                off += gsz
            nh = nb_b[E_G:]
            h_b = eye256[:, nh]  # [256, E_H] one-hot columns
            h_c[b, 0] = h_b[:128]
            h_c[b, 1] = h_b[128:]
        in_maps.append(
            {
                "x": x_c,
                "rbfT": rbfT_c,
                "idx": idx_c,
                "hmat": h_c,
                "wstack": ws,
                "bstack": bs,
                "k2fT": k2fT,
                "selbuf": selbuf,
                "ident": ident,
            }
        )
    return in_maps


def run(inputs, trace=False, **kwargs):
    global _GRAPH
    if _GRAPH is None:
        _GRAPH = build_graph()
    in_maps = _prep_core_inputs(inputs)
    res = run_bass_kernel_spmd(
        _GRAPH, in_maps, core_ids=list(range(NCORES)), trace=trace, **kwargs
    )
    outs = [np.asarray(res.results[i]["out"], np.float32) for i in range(NCORES)]
    full = np.concatenate(outs, axis=0)  # [B, N, F]
    return full, res


def kernel(**inputs):
    full, _ = run(inputs, trace=False)
    return full


# revision 82
# speedup vs baseline: 1.0136x; 1.0016x over previous
"""Trainium2 Bass kernel for AtomInteractionWithResidual (PhysNet-style GNN block).

Strategy (8 NeuronCores, data-parallel over batch B=32 -> 4 batches/core):
  - Host-side prep (layout only): rbf transposed to [K, edges] (bf16),
    MLP weights pre-transposed to lhsT form, softplus-shift (ln 2) folded
    into biases.
  - Neighbor-feature materialization is SPLIT per batch to balance the DMA
    engines against the PE array:
      * edges 0..9216   (T_G=72 tiles): dma_gather of softplus(x) rows from
        DRAM (bf16, 2 gather calls per batch)
      * edges 9216..16384 (T_H=56 tiles): one-hot matmul on the PE --
        xj_tile = H_tile.T @ xash where H is a host-built fp8 one-hot
        (exact 0/1 selection, 2 accumulating passes over the 256-atom
        contraction). H streams in as contiguous fp8 DMA, which is ~2x
        cheaper per edge than per-row gather descriptors. The resulting
        PSUM xj is evacuated to SBUF bf16 on the Act engine (DVE can only
        read one PSUM operand).
  - Device per batch:
      xa = softplus(x) - ln2 (bf16) -> DRAM (gather source) + SBUF (H rhs)
      g  = rbfT.T @ k2fT per 128-edge tile (PE)        [edge, F] f32 PSUM
      u  = g * xj (DVE)                                [edge, F] bf16
      sT += sel-window.T @ u (PE, accumulating)        [F, n]    feature-major
      feature-major MLP chain (interaction res blocks, gate, atom res blocks)
      using float32r matmuls (4x faster than f32 on the PE at 256-wide).
  - Scheduling structure (the cost model serializes all DMA on one device
    and every engine queue is in-order):
      * x->xa prologue for all batches runs first; compute-dependent writes
        (xa, out) issue on the Activation engine's HWDGE queue so they never
        head-of-line block the SP bulk-load queue.
      * PSUM pools use batch-parity tags (sT0/sT1, z0/z1) so adjacent
        batches' serial MLP chains get disjoint banks and can overlap.
      * Each batch's MLP chain is emitted as a generator advanced between
        the next batch's aggregation groups (program-order interleaving).
      * Batch 0 processes one-hot groups first (their inputs need no
        DRAM round-trip), starting the PE/DVE pipeline ~15us earlier.
"""

import numpy as np
import ml_dtypes
from contextlib import ExitStack

import concourse.bass as bass
from concourse import bacc
import concourse.mybir as mybir
import concourse.tile as tile
from concourse.bass_utils import run_bass_kernel_spmd

F32 = mybir.dt.float32
F32R = mybir.dt.float32r
BF16 = mybir.dt.bfloat16
FP8 = mybir.dt.float8e4
I16 = mybir.dt.int16
AF = mybir.ActivationFunctionType
ALU = mybir.AluOpType

B, N, M, F, K = 32, 256, 64, 128, 64
NCORES = 8
BPC = B // NCORES          # batches per core
E_B = N * M                # edges per batch (16384)
ET_B = E_B // 128          # 128-edge tiles per batch (128)
T_G = 56                   # tiles materialized via dma_gather
T_H = ET_B - T_G           # tiles materialized via one-hot matmul (72)
E_G = T_G * 128            # gathered edges per batch (7168)
E_H = T_H * 128            # one-hot edges per batch (9216)
G_SIZES = [E_G // 2, E_G // 2]  # gather call sizes
assert sum(G_SIZES) == E_G
GRP_SPLIT = T_G // 4       # group index where the H-sourced tiles start (14)
LN2 = float(np.log(2.0))

# weight stack order (lhsT = W.T each)
IW_WI = 0
IW_IRES = 1                # 1..6: (W1,W2) x 3
IW_WINT = 7
IW_ARES = 8                # 8..11: (W1,W2) x 2
# bias column order
IB_WI = 0
IB_IRES1 = 1               # 1..3
IB_IRES2 = 4               # 4..6
IB_WINT = 7
IB_ARES1 = 8               # 8..9
IB_ARES2 = 10              # 10..11
IB_UGATE = 12
IB_NLN2 = 13
IB_HALF = 14
NB = 15

_GRAPH = None


class _Bacc(bacc.Bacc):
    """Bacc with act-table preference reordered so the single table covering
    Exp+Ln+Copy (natural_log_exp_and_others) is picked for every activation,
    avoiding per-op table reload thrash."""

    def insert_act_table_loads(self):
        import concourse.mybir as _mb
        from concourse.hw_specs import get_activation_tables
        import bass_rust as _br

        has_activation = any(
            isinstance(i, _mb.InstActivation)
            for b in self.main_func.blocks
            for i in b.instructions
        )
        if not has_activation:
            return
        tables = [
            (name, s if name == "natural_log_exp_and_others" else set())
            for name, s in get_activation_tables(self.m.arch).items()
        ]
        _br.insert_act_table_loads(self, tables)


def build_graph():
    nc = _Bacc()

    x_in = nc.declare_dram_parameter("x", [BPC, N, F], F32, isOutput=False)
    rbfT_in = nc.declare_dram_parameter("rbfT", [K, BPC * E_B], BF16, isOutput=False)
    idx_in = nc.declare_dram_parameter("idx", [128, BPC * E_G // 16], I16, isOutput=False)
    h_in = nc.declare_dram_parameter("hmat", [BPC, 2, 128, E_H], FP8, isOutput=False)
    w_in = nc.declare_dram_parameter("wstack", [F, 12, F], F32R, isOutput=False)
    b_in = nc.declare_dram_parameter("bstack", [F, NB], F32, isOutput=False)
    k2fT_in = nc.declare_dram_parameter("k2fT", [K, F], BF16, isOutput=False)
    sel_in = nc.declare_dram_parameter("selbuf", [128, 66], BF16, isOutput=False)
    id_in = nc.declare_dram_parameter("ident", [128, 128], F32, isOutput=False)
    out_ext = nc.declare_dram_parameter("out", [BPC, N, F], F32, isOutput=True)

    with tile.TileContext(nc) as tc, ExitStack() as ctx:
        const = ctx.enter_context(tc.tile_pool(name="const", bufs=1))
        pro = ctx.enter_context(tc.tile_pool(name="pro", bufs=1))
        pairp = ctx.enter_context(tc.tile_pool(name="pairp", bufs=1))
        small = ctx.enter_context(tc.tile_pool(name="small", bufs=2))
        rbfp = ctx.enter_context(tc.tile_pool(name="rbfp", bufs=2))
        hp = ctx.enter_context(tc.tile_pool(name="hp", bufs=2))
        xjp = ctx.enter_context(tc.tile_pool(name="xjp", bufs=3))
        up = ctx.enter_context(tc.tile_pool(name="up", bufs=8))
        mlp = ctx.enter_context(tc.tile_pool(name="mlp", bufs=3))
        etp = ctx.enter_context(tc.tile_pool(name="etp", bufs=2))
        mlph = ctx.enter_context(tc.tile_pool(name="mlph", bufs=2))
        dramp = ctx.enter_context(tc.tile_pool(name="dramp", bufs=4, space="DRAM"))
        # PSUM budget: g(2 bufs)=2 banks, xh(2)=2, sT parity(1+1)=2,
        # z parity(1+1)=2 -> exactly 8 banks. Batch-parity tags decouple
        # adjacent batches' serial MLP chains (static round-robin buffer
        # assignment would otherwise serialize them).
        psg = ctx.enter_context(tc.tile_pool(name="psg", bufs=2, space="PSUM"))
        psh = ctx.enter_context(tc.tile_pool(name="psh", bufs=2, space="PSUM"))
        pss = ctx.enter_context(tc.tile_pool(name="pss", bufs=1, space="PSUM"))
        pz = ctx.enter_context(tc.tile_pool(name="pz", bufs=1, space="PSUM"))

        # constants (ordered so the gather-critical ones land first;
        # w_sb is only needed by the MLP and loads after batch 0's stream)
        idx_sb = const.tile([128, BPC * E_G // 16], I16)
        nc.sync.dma_start(out=idx_sb[:, :], in_=idx_in[:, :])
        b_sb = const.tile([F, NB], F32)
        nc.sync.dma_start(out=b_sb[:], in_=b_in[:, :])
        k2fT_sb = const.tile([K, F], BF16)
        nc.sync.dma_start(out=k2fT_sb[:], in_=k2fT_in[:, :])
        sel_sb = const.tile([128, 66], BF16)
        nc.sync.dma_start(out=sel_sb[:], in_=sel_in[:, :])
        ident = const.tile([128, 128], F32)
        nc.sync.dma_start(out=ident[:], in_=id_in[:, :])
        w_sb = const.tile([F, 12, F], F32R)

        def bias(col):
            return b_sb[:, col : col + 1]

        def softplus(dst, src, pre_bias, tmp_pool, tmp_tag):
            e = tmp_pool.tile(list(dst.shape), F32, tag=tmp_tag, name=f"e_{tmp_tag}")
            if pre_bias is None:
                nc.scalar.activation(e[:], src, AF.Exp)
            else:
                nc.scalar.activation(e[:], src, AF.Exp, bias=pre_bias)
            nc.scalar.activation(dst, e[:], AF.Ln, bias=1.0)

        def mm(out, lhsT, rhs, **kw):
            nc.tensor.matmul(out, lhsT, rhs, **kw)

        # ---- prologue: x load + softplus + gather-source write for ALL
        # batches up front, so the SP DMA queue is never head-of-line
        # blocked by a compute-dependent write mid-stream. The xa/out
        # writes go out on the Activation engine's HWDGE queue.
        xrows, xashs, xa_drams = [], [], []
        for b in range(BPC):
            xrow = pro.tile([128, 2, F], F32, tag=f"xrow{b}", name=f"xrow{b}")
            nc.sync.dma_start(
                out=xrow[:], in_=x_in[b].rearrange("(t p) f -> p t f", p=128)
            )
            # xash = softplus(x) - ln2 = Ln(exp(x)*0.5 + 0.5), bf16
            spe = small.tile([128, 2, F], F32, tag="spe")
            nc.scalar.activation(spe[:], xrow[:], AF.Exp)
            xash = pro.tile([128, 2, F], BF16, tag=f"xash{b}", name=f"xash{b}")
            nc.scalar.activation(
                xash[:], spe[:], AF.Ln, bias=bias(IB_HALF), scale=bias(IB_HALF)
            )
            xa_dram = dramp.tile([N, F], BF16, tag="xad")
            nc.scalar.dma_start(
                out=xa_dram[:].rearrange("(t p) f -> p t f", p=128), in_=xash[:]
            )
            xrows.append(xrow)
            xashs.append(xash)
            xa_drams.append(xa_dram)

        # MLP weights: not needed until the first batch's MLP, so load
        # after the prologue x traffic
        nc.sync.dma_start(out=w_sb[:], in_=w_in[:, :, :])

        # Per-batch MLP chains, emitted as GENERATORS advanced between the
        # next batch's aggregation groups. This interleaves the (serial,
        # latency-bound) chain ops with aggregation work in program order,
        # so the in-order engine queues never head-of-line block one stream
        # behind the other. Only the last batch's chain runs bare (the tail).
        def mlp_chain(b, P, xT, xi_sp, s_ps):
            def zt():
                return pz.tile([128, N], F32, tag=f"z{P}", name=f"z{P}")

            def sp(dst, src, pre_bias):
                e = etp.tile(list(dst.shape), F32, tag=f"et{P}", name=f"e{P}")
                if pre_bias is None:
                    nc.scalar.activation(e[:], src, AF.Exp)
                else:
                    nc.scalar.activation(e[:], src, AF.Exp, bias=pre_bias)
                yield
                nc.scalar.activation(dst, e[:], AF.Ln, bias=1.0)
                yield

            # ---- assemble h = xi + sT ------------------------------------
            s_sb = small.tile([128, 2, 128], F32, tag="ssb")
            sT2 = zt()
            for t in range(2):
                nc.scalar.activation(s_sb[:, t, :], s_ps[:, t, :], AF.Copy)
                yield
                nc.tensor.transpose(
                    sT2[:, t * 128 : (t + 1) * 128], s_sb[:, t, :], ident[:]
                )
                yield
            h = mlph.tile([128, N], F32, tag=f"h{P}")
            nc.vector.tensor_add(h[:], xi_sp[:], sT2[:])
            yield
            # ---- interaction res blocks (carrier h = v_true + ln2) -------
            for l in range(3):
                a1 = mlp.tile([128, N], F32R, tag=f"a1{P}")
                yield from sp(a1[:], h[:], bias(IB_NLN2))
                z1 = zt()
                mm(z1[:], w_sb[:, IW_IRES + 2 * l, :], a1[:], start=True, stop=True)
                yield
                a2 = mlp.tile([128, N], F32R, tag=f"a2{P}")
                yield from sp(a2[:], z1[:], bias(IB_IRES1 + l))
                z2 = zt()
                mm(z2[:], w_sb[:, IW_IRES + 2 * l + 1, :], a2[:], start=True, stop=True)
                yield
                h2 = mlph.tile([128, N], F32, tag=f"h{P}")
                nc.vector.scalar_tensor_tensor(
                    h2[:], z2[:], bias(IB_IRES2 + l), h[:], ALU.add, ALU.add
                )
                yield
                h = h2
            # ---- gate: out0 = u_gate*x + v @ Wint.T + bint_adj -----------
            av = mlp.tile([128, N], F32R, tag=f"a1{P}")
            yield from sp(av[:], h[:], bias(IB_NLN2))
            zv = zt()
            mm(zv[:], w_sb[:, IW_WINT, :], av[:], start=True, stop=True)
            yield
            gx = mlp.tile([128, N], F32, tag=f"a2{P}")
            nc.gpsimd.tensor_scalar_mul(
                gx[:], xT[:].rearrange("p t f -> p (t f)"), bias(IB_UGATE)
            )
            yield
            h = mlph.tile([128, N], F32, tag=f"h{P}")
            nc.vector.scalar_tensor_tensor(
                h[:], zv[:], bias(IB_WINT), gx[:], ALU.add, ALU.add
            )
            yield
            # ---- atom res blocks (true-valued carrier) -------------------
            for l in range(2):
                a1 = mlp.tile([128, N], F32R, tag=f"a1{P}")
                yield from sp(a1[:], h[:], None)
                z1 = zt()
                mm(z1[:], w_sb[:, IW_ARES + 2 * l, :], a1[:], start=True, stop=True)
                yield
                a2 = mlp.tile([128, N], F32R, tag=f"a2{P}")
                yield from sp(a2[:], z1[:], bias(IB_ARES1 + l))
                z2 = zt()
                mm(z2[:], w_sb[:, IW_ARES + 2 * l + 1, :], a2[:], start=True, stop=True)
                yield
                h2 = mlph.tile([128, N], F32, tag=f"h{P}")
                nc.vector.scalar_tensor_tensor(
                    h2[:], z2[:], bias(IB_ARES2 + l), h[:], ALU.add, ALU.add
                )
                yield
                h = h2
            # ---- output: transpose back to row-major ---------------------
            oT_ps = zt()
            o_sb = small.tile([128, 2, 128], F32, tag="osb")
            for t in range(2):
                nc.tensor.transpose(
                    oT_ps[:, t * 128 : (t + 1) * 128],
                    h[:, t * 128 : (t + 1) * 128],
                    ident[:],
                )
                yield
                nc.scalar.activation(
                    o_sb[:, t, :], oT_ps[:, t * 128 : (t + 1) * 128], AF.Copy
                )
                yield
                nc.scalar.dma_start(
                    out=out_ext[b, t * 128 : (t + 1) * 128, :], in_=o_sb[:, t, :]
                )
                yield

        pending = None
        for b in range(BPC):
            xrow, xash, xa_dram = xrows[b], xashs[b], xa_drams[b]
            P = b % 2

            # feature-major xT (for gate term) and sp(xT) (for xi matmul)
            xT = small.tile([128, 2, 128], F32, tag="xT")
            xaT = small.tile([128, 2, 128], F32R, tag="xaT")
            xT_ps = pz.tile([128, N], F32, tag=f"z{P}", name=f"z{P}")
            for t in range(2):
                nc.tensor.transpose(
                    xT_ps[:, t * 128 : (t + 1) * 128], xrow[:, t, :], ident[:]
                )
                nc.scalar.activation(
                    xT[:, t, :], xT_ps[:, t * 128 : (t + 1) * 128], AF.Copy
                )
                softplus(
                    xaT[:, t, :], xT_ps[:, t * 128 : (t + 1) * 128], None,
                    small, "xaTe",
                )

            # xi = softplus(zi + bi_adj); carrier v+ln2 = xi_sp + sT
            zi_ps = pz.tile([128, N], F32, tag=f"z{P}", name=f"z{P}")
            mm(
                zi_ps[:],
                w_sb[:, IW_WI, :],
                xaT[:].rearrange("p t f -> p (t f)"),
                start=True,
                stop=True,
            )
            xi_sp = mlp.tile([128, N], F32, tag=f"xi{P}")
            softplus(xi_sp[:], zi_ps[:], bias(IB_WI), etp, f"et{P}")

            # ---- gather + one-hot H + g + u + reduce --------------------
            xj = xjp.tile([128, T_G, F], BF16, tag="xj")
            off = 0
            for c, gsz in enumerate(G_SIZES):
                col0 = b * (E_G // 16) + off // 16
                nc.gpsimd.dma_gather(
                    out_ap=xj[:, off // 128 : (off + gsz) // 128, :],
                    in_ap=xa_dram[:, :],
                    idxs_ap=idx_sb[:, col0 : col0 + gsz // 16],
                    num_idxs=gsz,
                    num_idxs_reg=gsz,
                    elem_size=F,
                    single_packet=False,
                )
                off += gsz

            # H-sourced groups are processed FIRST (their inputs need no
            # gather round-trip), so load H + the back rbf quarters first;
            # the gather lands under the H-group compute.
            rbfT_sb = rbfp.tile([K, E_B], BF16, tag="rbfT")
            h_sb = hp.tile([128, 2, E_H], FP8, tag="hmat")
            EQ = E_B // 4

            def load_rbf_q(q):
                nc.sync.dma_start(
                    out=rbfT_sb[:, q * EQ : (q + 1) * EQ],
                    in_=rbfT_in[:, b * E_B + q * EQ : b * E_B + (q + 1) * EQ],
                )

            def load_h_half(hh):
                nc.sync.dma_start(
                    out=h_sb[:, :, hh * (E_H // 2) : (hh + 1) * (E_H // 2)],
                    in_=h_in[
                        b, :, :, hh * (E_H // 2) : (hh + 1) * (E_H // 2)
                    ].rearrange("h p e -> p h e"),
                )

            if b == 0:
                # batch 0: H inputs first so the PE/DVE pipeline starts
                # ~15us earlier (H-groups need no gather round-trip)
                load_rbf_q(2)
                load_h_half(0)
                load_rbf_q(3)
                load_h_half(1)
            else:
                load_rbf_q(0)
                load_rbf_q(1)
                load_h_half(0)

            s_ps = pss.tile([128, 2, 128], F32, tag=f"sT{P}", name=f"sT{P}")
            if b == 0:
                # H-groups 20..31 first (whole 16-tile accumulation blocks
                # only -- one pending PSUM group at a time), then 0..19
                grp_order = list(range(GRP_SPLIT + 6, ET_B // 4)) + list(
                    range(GRP_SPLIT + 6)
                )
                deferred_loads = {2: lambda: load_rbf_q(0),
                                  6: lambda: load_rbf_q(1)}
            else:
                grp_order = list(range(ET_B // 4))
                deferred_loads = {3: lambda: load_rbf_q(2),
                                  6: lambda: (load_rbf_q(3), load_h_half(1))}
            tile_pos = {}
            for _i, _g in enumerate(grp_order):
                for _j in range(4):
                    tile_pos[_g * 4 + _j] = _i * 4 + _j
            for gi, grp in enumerate(grp_order):  # 32 groups of 4 tiles
                if gi in deferred_loads:
                    deferred_loads[gi]()
                g_ps = psg.tile([128, 4, 128], F32, tag="g")
                for j in range(4):
                    t = grp * 4 + j
                    nc.tensor.matmul(
                        g_ps[:, j, :],
                        rbfT_sb[:, t * 128 : (t + 1) * 128],
                        k2fT_sb[:],
                        start=True,
                        stop=True,
                    )
                u = up.tile([128, 4, 128], BF16, tag="u")
                if grp < GRP_SPLIT:
                    if b < 1:
                        # early batches: Act is idle here, so evacuate g to
                        # bf16 SBUF and run the multiply in DVE 2x mode
                        # (all-2-byte operands)
                        g_sb = up.tile([128, 4, 128], BF16, tag="gsb")
                        nc.scalar.activation(
                            g_sb[:].rearrange("p a f -> p (a f)"),
                            g_ps[:].rearrange("p a f -> p (a f)"),
                            AF.Copy,
                        )
                        nc.vector.tensor_mul(
                            u[:].rearrange("p a f -> p (a f)"),
                            g_sb[:].rearrange("p a f -> p (a f)"),
                            xj[:, grp * 4 : (grp + 1) * 4, :].rearrange(
                                "p a f -> p (a f)"
                            ),
                        )
                    else:
                        nc.vector.tensor_mul(
                            u[:].rearrange("p a f -> p (a f)"),
                            g_ps[:].rearrange("p a f -> p (a f)"),
                            xj[:, grp * 4 : (grp + 1) * 4, :].rearrange(
                                "p a f -> p (a f)"
                            ),
                        )
                else:
                    # one-hot-sourced tiles: xj = H_tile.T @ xash in PSUM.
                    # DVE can read only one PSUM operand, so evacuate xh to
                    # SBUF (bf16) on the Act engine before the multiply.
                    xh_ps = psh.tile([128, 4, 128], F32, tag="xh")
                    for j in range(4):
                        e0 = (grp - GRP_SPLIT) * 4 * 128 + j * 128
                        nc.tensor.matmul(
                            xh_ps[:, j, :],
                            h_sb[:, 0, e0 : e0 + 128],
                            xash[:, 0, :],
                            start=True,
                            stop=False,
                        )
                        nc.tensor.matmul(
                            xh_ps[:, j, :],
                            h_sb[:, 1, e0 : e0 + 128],
                            xash[:, 1, :],
                            start=False,
                            stop=True,
                        )
                    xh_sb = up.tile([128, 4, 128], BF16, tag="xhs")
                    nc.scalar.activation(
                        xh_sb[:].rearrange("p a f -> p (a f)"),
                        xh_ps[:].rearrange("p a f -> p (a f)"),
                        AF.Copy,
                    )
                    if b < 1:
                        g_sb = up.tile([128, 4, 128], BF16, tag="gsb")
                        nc.scalar.activation(
                            g_sb[:].rearrange("p a f -> p (a f)"),
                            g_ps[:].rearrange("p a f -> p (a f)"),
                            AF.Copy,
                        )
                        nc.vector.tensor_mul(
                            u[:].rearrange("p a f -> p (a f)"),
                            g_sb[:].rearrange("p a f -> p (a f)"),
                            xh_sb[:].rearrange("p a f -> p (a f)"),
                        )
                    else:
                        nc.vector.tensor_mul(
                            u[:].rearrange("p a f -> p (a f)"),
                            g_ps[:].rearrange("p a f -> p (a f)"),
                            xh_sb[:].rearrange("p a f -> p (a f)"),
                        )
                for j in range(4):
                    t = grp * 4 + j
                    # edge-tile t -> n-pair (2t, 2t+1); s-tile half = t // 64;
                    # 32-partition block bblk = (t % 64) // 16; slot r = t % 16.
                    # start/stop follow PROCESSING order (H-groups run first,
                    # so a block straddling the gather/H boundary starts at
                    # its first-processed tile).
                    half = t // 64
                    bblk = (t % 64) // 16
                    r = t % 16
                    blk0 = (t // 16) * 16
                    pos = [tile_pos[blk0 + rr] for rr in range(16)]
                    nc.tensor.matmul(
                        s_ps[32 * bblk : 32 * bblk + 32, half, :],
                        sel_sb[:, 32 - 2 * r : 64 - 2 * r],
                        u[:, j, :],
                        start=(tile_pos[t] == min(pos)),
                        stop=(tile_pos[t] == max(pos)),
                        tile_position=(0, 32 * bblk),
                    )
                # advance the previous batch's MLP chain a couple of ops
                if pending is not None:
                    for _ in range(2):
                        if next(pending, "done") == "done":
                            pending = None
                            break

            if pending is not None:
                for _ in pending:
                    pass
            pending = mlp_chain(b, P, xT, xi_sp, s_ps)

        if pending is not None:
            for _ in pending:
                pass

    nc.compile()
    return nc


def _prep_core_inputs(inputs):
    """Host-side layout prep. Returns in_maps for the 8 cores."""
    x = np.asarray(inputs["x"], np.float32)
    rbf = np.asarray(inputs["rbf"], np.float32)
    neighbor = np.asarray(inputs["neighbor"])
    k2f_W = np.asarray(inputs["k2f_W"], np.float32)

    c = LN2

    def lhsT(w):
        return np.ascontiguousarray(np.asarray(w, np.float32).T)

    # weight stack [F, 12, F]
    ws = np.zeros((F, 12, F), np.float32)
    ws[:, IW_WI, :] = lhsT(inputs["Wi"])
    for l in range(3):
        ws[:, IW_IRES + 2 * l, :] = lhsT(inputs["ires_W1"][l])
        ws[:, IW_IRES + 2 * l + 1, :] = lhsT(inputs["ires_W2"][l])
    ws[:, IW_WINT, :] = lhsT(inputs["Wint"])
    for l in range(2):
        ws[:, IW_ARES + 2 * l, :] = lhsT(inputs["ares_W1"][l])
        ws[:, IW_ARES + 2 * l + 1, :] = lhsT(inputs["ares_W2"][l])

    # bias stack [F, NB] (softplus shift folded in)
    bs = np.zeros((F, NB), np.float32)
    rs = lambda w: np.asarray(w, np.float32).sum(axis=1)
    bs[:, IB_NLN2] = -c
    bs[:, IB_HALF] = 0.5
    bs[:, IB_UGATE] = np.asarray(inputs["u_gate"], np.float32)
    bs[:, IB_WI] = inputs["bi"] - c * rs(inputs["Wi"])
    for l in range(3):
        bs[:, IB_IRES1 + l] = inputs["ires_b1"][l] - c * rs(inputs["ires_W1"][l])
        bs[:, IB_IRES2 + l] = inputs["ires_b2"][l] - c * rs(inputs["ires_W2"][l])
    bs[:, IB_WINT] = inputs["bint"] - c * rs(inputs["Wint"])
    for l in range(2):
        bs[:, IB_ARES1 + l] = inputs["ares_b1"][l] - c * rs(inputs["ares_W1"][l])
        bs[:, IB_ARES2 + l] = inputs["ares_b2"][l] - c * rs(inputs["ares_W2"][l])

    k2fT = np.ascontiguousarray(k2f_W.T).astype(ml_dtypes.bfloat16)  # [K, F]

    selbuf = np.zeros((128, 66), ml_dtypes.bfloat16)
    selbuf[:64, 32] = 1
    selbuf[64:, 33] = 1

    ident = np.eye(128, dtype=np.float32)

    in_maps = []
    eye256 = np.eye(256, dtype=ml_dtypes.float8_e4m3fn)
    for i in range(NCORES):
        bs_lo = i * BPC
        x_c = np.ascontiguousarray(x[bs_lo : bs_lo + BPC])
        rbf_c = rbf[bs_lo : bs_lo + BPC].reshape(BPC * E_B, K)
        rbfT_c = np.ascontiguousarray(rbf_c.T).astype(ml_dtypes.bfloat16)
        nbr_c = neighbor[bs_lo : bs_lo + BPC].reshape(BPC, E_B).astype(np.int64)
        # gather idxs: first E_G edges of each batch.
        # dma_gather wrap: idx i -> partition i%16, col i//16, per call
        idx_c = np.zeros((128, BPC * E_G // 16), np.int16)
        # one-hot H for the last E_H edges: [BPC, 2, 128, E_H]
        h_c = np.zeros((BPC, 2, 128, E_H), ml_dtypes.float8_e4m3fn)
        for b in range(BPC):
            nb_b = nbr_c[b]
            ng = nb_b[:E_G].astype(np.int16)
            off = 0
            for gsz in G_SIZES:
                seg = ng[off : off + gsz]
                wrap = np.tile(seg.reshape(gsz // 16, 16).T, (8, 1))
                col0 = b * (E_G // 16) + off // 16
                idx_c[:, col0 : col0 + gsz // 16] = wrap
                off += gsz
            nh = nb_b[E_G:]
            h_b = eye256[:, nh]  # [256, E_H] one-hot columns
            h_c[b, 0] = h_b[:128]
            h_c[b, 1] = h_b[128:]
        in_maps.append(
            {
                "x": x_c,
                "rbfT": rbfT_c,
                "idx": idx_c,
                "hmat": h_c,
                "wstack": ws,
                "bstack": bs,
                "k2fT": k2fT,
                "selbuf": selbuf,
                "ident": ident,
            }
        )
    return in_maps


def run(inputs, trace=False, **kwargs):
    global _GRAPH
    if _GRAPH is None:
        _GRAPH = build_graph()
    in_maps = _prep_core_inputs(inputs)
    res = run_bass_kernel_spmd(
        _GRAPH, in_maps, core_ids=list(range(NCORES)), trace=trace, **kwargs
    )
    outs = [np.asarray(res.results[i]["out"], np.float32) for i in range(NCORES)]
    full = np.concatenate(outs, axis=0)  # [B, N, F]
    return full, res


def kernel(**inputs):
    full, _ = run(inputs, trace=False)
    return full


# revision 85
# speedup vs baseline: 1.0209x; 1.0072x over previous
"""Trainium2 Bass kernel for AtomInteractionWithResidual (PhysNet-style GNN block).

Strategy (8 NeuronCores, data-parallel over batch B=32 -> 4 batches/core):
  - Host-side prep (layout only): rbf transposed to [K, edges] (bf16),
    MLP weights pre-transposed to lhsT form, softplus-shift (ln 2) folded
    into biases.
  - Neighbor-feature materialization is SPLIT per batch to balance the DMA
    engines against the PE array:
      * edges 0..9216   (T_G=72 tiles): dma_gather of softplus(x) rows from
        DRAM (bf16, 2 gather calls per batch)
      * edges 9216..16384 (T_H=56 tiles): one-hot matmul on the PE --
        xj_tile = H_tile.T @ xash where H is a host-built fp8 one-hot
        (exact 0/1 selection, 2 accumulating passes over the 256-atom
        contraction). H streams in as contiguous fp8 DMA, which is ~2x
        cheaper per edge than per-row gather descriptors. The resulting
        PSUM xj is evacuated to SBUF bf16 on the Act engine (DVE can only
        read one PSUM operand).
  - Device per batch:
      xa = softplus(x) - ln2 (bf16) -> DRAM (gather source) + SBUF (H rhs)
      g  = rbfT.T @ k2fT per 128-edge tile (PE)        [edge, F] f32 PSUM
      u  = g * xj (DVE)                                [edge, F] bf16
      sT += sel-window.T @ u (PE, accumulating)        [F, n]    feature-major
      feature-major MLP chain (interaction res blocks, gate, atom res blocks)
      using float32r matmuls (4x faster than f32 on the PE at 256-wide).
  - Scheduling structure (the cost model serializes all DMA on one device
    and every engine queue is in-order):
      * x->xa prologue for all batches runs first; compute-dependent writes
        (xa, out) issue on the Activation engine's HWDGE queue so they never
        head-of-line block the SP bulk-load queue.
      * PSUM pools use batch-parity tags (sT0/sT1, z0/z1) so adjacent
        batches' serial MLP chains get disjoint banks and can overlap.
      * Each batch's MLP chain is emitted as a generator advanced between
        the next batch's aggregation groups (program-order interleaving).
      * Batch 0 processes one-hot groups first (their inputs need no
        DRAM round-trip), starting the PE/DVE pipeline ~15us earlier.
"""

import numpy as np
import ml_dtypes
from contextlib import ExitStack

import concourse.bass as bass
from concourse import bacc
import concourse.mybir as mybir
import concourse.tile as tile
from concourse.bass_utils import run_bass_kernel_spmd

F32 = mybir.dt.float32
F32R = mybir.dt.float32r
BF16 = mybir.dt.bfloat16
FP8 = mybir.dt.float8e4
I16 = mybir.dt.int16
AF = mybir.ActivationFunctionType
ALU = mybir.AluOpType

B, N, M, F, K = 32, 256, 64, 128, 64
NCORES = 8
BPC = B // NCORES          # batches per core
E_B = N * M                # edges per batch (16384)
ET_B = E_B // 128          # 128-edge tiles per batch (128)
T_G = 56                   # tiles materialized via dma_gather
T_H = ET_B - T_G           # tiles materialized via one-hot matmul (72)
E_G = T_G * 128            # gathered edges per batch (7168)
E_H = T_H * 128            # one-hot edges per batch (9216)
G_SIZES = [E_G - 4096, 4096]  # gather call sizes
assert sum(G_SIZES) == E_G
GRP_SPLIT = T_G // 4       # group index where the H-sourced tiles start (14)
LN2 = float(np.log(2.0))

# weight stack order (lhsT = W.T each)
IW_WI = 0
IW_IRES = 1                # 1..6: (W1,W2) x 3
IW_WINT = 7
IW_ARES = 8                # 8..11: (W1,W2) x 2
# bias column order
IB_WI = 0
IB_IRES1 = 1               # 1..3
IB_IRES2 = 4               # 4..6
IB_WINT = 7
IB_ARES1 = 8               # 8..9
IB_ARES2 = 10              # 10..11
IB_UGATE = 12
IB_NLN2 = 13
IB_HALF = 14
NB = 15

_GRAPH = None


class _Bacc(bacc.Bacc):
    """Bacc with act-table preference reordered so the single table covering
    Exp+Ln+Copy (natural_log_exp_and_others) is picked for every activation,
    avoiding per-op table reload thrash."""

    def insert_act_table_loads(self):
        import concourse.mybir as _mb
        from concourse.hw_specs import get_activation_tables
        import bass_rust as _br

        has_activation = any(
            isinstance(i, _mb.InstActivation)
            for b in self.main_func.blocks
            for i in b.instructions
        )
        if not has_activation:
            return
        tables = [
            (name, s if name == "natural_log_exp_and_others" else set())
            for name, s in get_activation_tables(self.m.arch).items()
        ]
        _br.insert_act_table_loads(self, tables)


def build_graph():
    nc = _Bacc()

    x_in = nc.declare_dram_parameter("x", [BPC, N, F], F32, isOutput=False)
    rbfT_in = nc.declare_dram_parameter("rbfT", [K, BPC * E_B], BF16, isOutput=False)
    idx_in = nc.declare_dram_parameter("idx", [128, BPC * E_G // 16], I16, isOutput=False)
    h_in = nc.declare_dram_parameter("hmat", [BPC, 2, 128, E_H], FP8, isOutput=False)
    w_in = nc.declare_dram_parameter("wstack", [F, 12, F], F32R, isOutput=False)
    b_in = nc.declare_dram_parameter("bstack", [F, NB], F32, isOutput=False)
    k2fT_in = nc.declare_dram_parameter("k2fT", [K, F], BF16, isOutput=False)
    sel_in = nc.declare_dram_parameter("selbuf", [128, 66], BF16, isOutput=False)
    id_in = nc.declare_dram_parameter("ident", [128, 128], F32, isOutput=False)
    out_ext = nc.declare_dram_parameter("out", [BPC, N, F], F32, isOutput=True)

    with tile.TileContext(nc) as tc, ExitStack() as ctx:
        const = ctx.enter_context(tc.tile_pool(name="const", bufs=1))
        pro = ctx.enter_context(tc.tile_pool(name="pro", bufs=1))
        pairp = ctx.enter_context(tc.tile_pool(name="pairp", bufs=1))
        small = ctx.enter_context(tc.tile_pool(name="small", bufs=2))
        rbfp = ctx.enter_context(tc.tile_pool(name="rbfp", bufs=2))
        hp = ctx.enter_context(tc.tile_pool(name="hp", bufs=2))
        xjp = ctx.enter_context(tc.tile_pool(name="xjp", bufs=3))
        up = ctx.enter_context(tc.tile_pool(name="up", bufs=8))
        mlp = ctx.enter_context(tc.tile_pool(name="mlp", bufs=3))
        etp = ctx.enter_context(tc.tile_pool(name="etp", bufs=2))
        mlph = ctx.enter_context(tc.tile_pool(name="mlph", bufs=2))
        dramp = ctx.enter_context(tc.tile_pool(name="dramp", bufs=4, space="DRAM"))
        # PSUM budget: g(2 bufs)=2 banks, xh(2)=2, sT parity(1+1)=2,
        # z parity(1+1)=2 -> exactly 8 banks. Batch-parity tags decouple
        # adjacent batches' serial MLP chains (static round-robin buffer
        # assignment would otherwise serialize them).
        psg = ctx.enter_context(tc.tile_pool(name="psg", bufs=2, space="PSUM"))
        psh = ctx.enter_context(tc.tile_pool(name="psh", bufs=2, space="PSUM"))
        pss = ctx.enter_context(tc.tile_pool(name="pss", bufs=1, space="PSUM"))
        pz = ctx.enter_context(tc.tile_pool(name="pz", bufs=1, space="PSUM"))

        # constants (ordered so the gather-critical ones land first;
        # w_sb is only needed by the MLP and loads after batch 0's stream)
        idx_sb = const.tile([128, BPC * E_G // 16], I16)
        nc.sync.dma_start(out=idx_sb[:, :], in_=idx_in[:, :])
        b_sb = const.tile([F, NB], F32)
        nc.sync.dma_start(out=b_sb[:], in_=b_in[:, :])
        k2fT_sb = const.tile([K, F], BF16)
        nc.sync.dma_start(out=k2fT_sb[:], in_=k2fT_in[:, :])
        sel_sb = const.tile([128, 66], BF16)
        nc.sync.dma_start(out=sel_sb[:], in_=sel_in[:, :])
        ident = const.tile([128, 128], F32)
        nc.sync.dma_start(out=ident[:], in_=id_in[:, :])
        w_sb = const.tile([F, 12, F], F32R)

        def bias(col):
            return b_sb[:, col : col + 1]

        def softplus(dst, src, pre_bias, tmp_pool, tmp_tag):
            e = tmp_pool.tile(list(dst.shape), F32, tag=tmp_tag, name=f"e_{tmp_tag}")
            if pre_bias is None:
                nc.scalar.activation(e[:], src, AF.Exp)
            else:
                nc.scalar.activation(e[:], src, AF.Exp, bias=pre_bias)
            nc.scalar.activation(dst, e[:], AF.Ln, bias=1.0)

        def mm(out, lhsT, rhs, **kw):
            nc.tensor.matmul(out, lhsT, rhs, **kw)

        # ---- prologue: x load + softplus + gather-source write for ALL
        # batches up front, so the SP DMA queue is never head-of-line
        # blocked by a compute-dependent write mid-stream. The xa/out
        # writes go out on the Activation engine's HWDGE queue.
        xrows, xashs, xa_drams = [], [], []
        for b in range(BPC):
            xrow = pro.tile([128, 2, F], F32, tag=f"xrow{b}", name=f"xrow{b}")
            nc.sync.dma_start(
                out=xrow[:], in_=x_in[b].rearrange("(t p) f -> p t f", p=128)
            )
            # xash = softplus(x) - ln2 = Ln(exp(x)*0.5 + 0.5), bf16
            spe = small.tile([128, 2, F], F32, tag="spe")
            nc.scalar.activation(spe[:], xrow[:], AF.Exp)
            xash = pro.tile([128, 2, F], BF16, tag=f"xash{b}", name=f"xash{b}")
            nc.scalar.activation(
                xash[:], spe[:], AF.Ln, bias=bias(IB_HALF), scale=bias(IB_HALF)
            )
            xa_dram = dramp.tile([N, F], BF16, tag="xad")
            nc.scalar.dma_start(
                out=xa_dram[:].rearrange("(t p) f -> p t f", p=128), in_=xash[:]
            )
            xrows.append(xrow)
            xashs.append(xash)
            xa_drams.append(xa_dram)

        # MLP weights: not needed until the first batch's MLP, so load
        # after the prologue x traffic
        nc.sync.dma_start(out=w_sb[:], in_=w_in[:, :, :])

        # Per-batch MLP chains, emitted as GENERATORS advanced between the
        # next batch's aggregation groups. This interleaves the (serial,
        # latency-bound) chain ops with aggregation work in program order,
        # so the in-order engine queues never head-of-line block one stream
        # behind the other. Only the last batch's chain runs bare (the tail).
        def mlp_chain(b, P, xT, xi_sp, s_ps):
            def zt():
                return pz.tile([128, N], F32, tag=f"z{P}", name=f"z{P}")

            def sp(dst, src, pre_bias):
                e = etp.tile(list(dst.shape), F32, tag=f"et{P}", name=f"e{P}")
                if pre_bias is None:
                    nc.scalar.activation(e[:], src, AF.Exp)
                else:
                    nc.scalar.activation(e[:], src, AF.Exp, bias=pre_bias)
                yield
                nc.scalar.activation(dst, e[:], AF.Ln, bias=1.0)
                yield

            # ---- assemble h = xi + sT ------------------------------------
            s_sb = small.tile([128, 2, 128], F32, tag="ssb")
            sT2 = zt()
            for t in range(2):
                nc.scalar.activation(s_sb[:, t, :], s_ps[:, t, :], AF.Copy)
                yield
                nc.tensor.transpose(
                    sT2[:, t * 128 : (t + 1) * 128], s_sb[:, t, :], ident[:]
                )
                yield
            h = mlph.tile([128, N], F32, tag=f"h{P}")
            nc.vector.tensor_add(h[:], xi_sp[:], sT2[:])
            yield
            # ---- interaction res blocks (carrier h = v_true + ln2) -------
            for l in range(3):
                a1 = mlp.tile([128, N], F32R, tag=f"a1{P}")
                yield from sp(a1[:], h[:], bias(IB_NLN2))
                z1 = zt()
                mm(z1[:], w_sb[:, IW_IRES + 2 * l, :], a1[:], start=True, stop=True)
                yield
                a2 = mlp.tile([128, N], F32R, tag=f"a2{P}")
                yield from sp(a2[:], z1[:], bias(IB_IRES1 + l))
                z2 = zt()
                mm(z2[:], w_sb[:, IW_IRES + 2 * l + 1, :], a2[:], start=True, stop=True)
                yield
                h2 = mlph.tile([128, N], F32, tag=f"h{P}")
                nc.vector.scalar_tensor_tensor(
                    h2[:], z2[:], bias(IB_IRES2 + l), h[:], ALU.add, ALU.add
                )
                yield
                h = h2
            # ---- gate: out0 = u_gate*x + v @ Wint.T + bint_adj -----------
            av = mlp.tile([128, N], F32R, tag=f"a1{P}")
            yield from sp(av[:], h[:], bias(IB_NLN2))
            zv = zt()
            mm(zv[:], w_sb[:, IW_WINT, :], av[:], start=True, stop=True)
            yield
            gx = mlp.tile([128, N], F32, tag=f"a2{P}")
            nc.gpsimd.tensor_scalar_mul(
                gx[:], xT[:].rearrange("p t f -> p (t f)"), bias(IB_UGATE)
            )
            yield
            h = mlph.tile([128, N], F32, tag=f"h{P}")
            nc.vector.scalar_tensor_tensor(
                h[:], zv[:], bias(IB_WINT), gx[:], ALU.add, ALU.add
            )
            yield
            # ---- atom res blocks (true-valued carrier) -------------------
            for l in range(2):
                a1 = mlp.tile([128, N], F32R, tag=f"a1{P}")
                yield from sp(a1[:], h[:], None)
                z1 = zt()
                mm(z1[:], w_sb[:, IW_ARES + 2 * l, :], a1[:], start=True, stop=True)
                yield
                a2 = mlp.tile([128, N], F32R, tag=f"a2{P}")
                yield from sp(a2[:], z1[:], bias(IB_ARES1 + l))
                z2 = zt()
                mm(z2[:], w_sb[:, IW_ARES + 2 * l + 1, :], a2[:], start=True, stop=True)
                yield
                h2 = mlph.tile([128, N], F32, tag=f"h{P}")
                nc.vector.scalar_tensor_tensor(
                    h2[:], z2[:], bias(IB_ARES2 + l), h[:], ALU.add, ALU.add
                )
                yield
                h = h2
            # ---- output: transpose back to row-major ---------------------
            oT_ps = zt()
            o_sb = small.tile([128, 2, 128], F32, tag="osb")
            for t in range(2):
                nc.tensor.transpose(
                    oT_ps[:, t * 128 : (t + 1) * 128],
                    h[:, t * 128 : (t + 1) * 128],
                    ident[:],
                )
                yield
                nc.scalar.activation(
                    o_sb[:, t, :], oT_ps[:, t * 128 : (t + 1) * 128], AF.Copy
                )
                yield
                nc.scalar.dma_start(
                    out=out_ext[b, t * 128 : (t + 1) * 128, :], in_=o_sb[:, t, :]
                )
                yield

        pending = None
        for b in range(BPC):
            xrow, xash, xa_dram = xrows[b], xashs[b], xa_drams[b]
            P = b % 2

            # feature-major xT (for gate term) and sp(xT) (for xi matmul)
            xT = small.tile([128, 2, 128], F32, tag="xT")
            xaT = small.tile([128, 2, 128], F32R, tag="xaT")
            xT_ps = pz.tile([128, N], F32, tag=f"z{P}", name=f"z{P}")
            for t in range(2):
                nc.tensor.transpose(
                    xT_ps[:, t * 128 : (t + 1) * 128], xrow[:, t, :], ident[:]
                )
                nc.scalar.activation(
                    xT[:, t, :], xT_ps[:, t * 128 : (t + 1) * 128], AF.Copy
                )
                softplus(
                    xaT[:, t, :], xT_ps[:, t * 128 : (t + 1) * 128], None,
                    small, "xaTe",
                )

            # xi = softplus(zi + bi_adj); carrier v+ln2 = xi_sp + sT
            zi_ps = pz.tile([128, N], F32, tag=f"z{P}", name=f"z{P}")
            mm(
                zi_ps[:],
                w_sb[:, IW_WI, :],
                xaT[:].rearrange("p t f -> p (t f)"),
                start=True,
                stop=True,
            )
            xi_sp = mlp.tile([128, N], F32, tag=f"xi{P}")
            softplus(xi_sp[:], zi_ps[:], bias(IB_WI), etp, f"et{P}")

            # ---- gather + one-hot H + g + u + reduce --------------------
            xj = xjp.tile([128, T_G, F], BF16, tag="xj")
            off = 0
            for c, gsz in enumerate(G_SIZES):
                col0 = b * (E_G // 16) + off // 16
                nc.gpsimd.dma_gather(
                    out_ap=xj[:, off // 128 : (off + gsz) // 128, :],
                    in_ap=xa_dram[:, :],
                    idxs_ap=idx_sb[:, col0 : col0 + gsz // 16],
                    num_idxs=gsz,
                    num_idxs_reg=gsz,
                    elem_size=F,
                    single_packet=False,
                )
                off += gsz

            # H-sourced groups are processed FIRST (their inputs need no
            # gather round-trip), so load H + the back rbf quarters first;
            # the gather lands under the H-group compute.
            rbfT_sb = rbfp.tile([K, E_B], BF16, tag="rbfT")
            h_sb = hp.tile([128, 2, E_H], FP8, tag="hmat")
            EQ = E_B // 4

            def load_rbf_q(q):
                nc.sync.dma_start(
                    out=rbfT_sb[:, q * EQ : (q + 1) * EQ],
                    in_=rbfT_in[:, b * E_B + q * EQ : b * E_B + (q + 1) * EQ],
                )

            def load_h_half(hh):
                nc.sync.dma_start(
                    out=h_sb[:, :, hh * (E_H // 2) : (hh + 1) * (E_H // 2)],
                    in_=h_in[
                        b, :, :, hh * (E_H // 2) : (hh + 1) * (E_H // 2)
                    ].rearrange("h p e -> p h e"),
                )

            if b == 0:
                # batch 0: H inputs first so the PE/DVE pipeline starts
                # ~15us earlier (H-groups need no gather round-trip)
                load_rbf_q(2)
                load_h_half(0)
                load_rbf_q(3)
                load_h_half(1)
            else:
                load_rbf_q(0)
                load_rbf_q(1)
                load_h_half(0)

            s_ps = pss.tile([128, 2, 128], F32, tag=f"sT{P}", name=f"sT{P}")
            if b == 0:
                # H-groups 20..31 first (whole 16-tile accumulation blocks
                # only -- one pending PSUM group at a time), then 0..19
                grp_order = list(range(GRP_SPLIT + 6, ET_B // 4)) + list(
                    range(GRP_SPLIT + 6)
                )
                deferred_loads = {2: lambda: load_rbf_q(0),
                                  6: lambda: load_rbf_q(1)}
            else:
                grp_order = list(range(ET_B // 4))
                deferred_loads = {3: lambda: load_rbf_q(2),
                                  6: lambda: (load_rbf_q(3), load_h_half(1))}
            tile_pos = {}
            for _i, _g in enumerate(grp_order):
                for _j in range(4):
                    tile_pos[_g * 4 + _j] = _i * 4 + _j
            for gi, grp in enumerate(grp_order):  # 32 groups of 4 tiles
                if gi in deferred_loads:
                    deferred_loads[gi]()
                g_ps = psg.tile([128, 4, 128], F32, tag="g")
                for j in range(4):
                    t = grp * 4 + j
                    nc.tensor.matmul(
                        g_ps[:, j, :],
                        rbfT_sb[:, t * 128 : (t + 1) * 128],
                        k2fT_sb[:],
                        start=True,
                        stop=True,
                    )
                u = up.tile([128, 4, 128], BF16, tag="u")
                if grp < GRP_SPLIT:
                    if b < 1:
                        # early batches: Act is idle here, so evacuate g to
                        # bf16 SBUF and run the multiply in DVE 2x mode
                        # (all-2-byte operands)
                        g_sb = up.tile([128, 4, 128], BF16, tag="gsb")
                        nc.scalar.activation(
                            g_sb[:].rearrange("p a f -> p (a f)"),
                            g_ps[:].rearrange("p a f -> p (a f)"),
                            AF.Copy,
                        )
                        nc.vector.tensor_mul(
                            u[:].rearrange("p a f -> p (a f)"),
                            g_sb[:].rearrange("p a f -> p (a f)"),
                            xj[:, grp * 4 : (grp + 1) * 4, :].rearrange(
                                "p a f -> p (a f)"
                            ),
                        )
                    else:
                        nc.vector.tensor_mul(
                            u[:].rearrange("p a f -> p (a f)"),
                            g_ps[:].rearrange("p a f -> p (a f)"),
                            xj[:, grp * 4 : (grp + 1) * 4, :].rearrange(
                                "p a f -> p (a f)"
                            ),
                        )
                else:
                    # one-hot-sourced tiles: xj = H_tile.T @ xash in PSUM.
                    # DVE can read only one PSUM operand, so evacuate xh to
                    # SBUF (bf16) on the Act engine before the multiply.
                    xh_ps = psh.tile([128, 4, 128], F32, tag="xh")
                    for j in range(4):
                        e0 = (grp - GRP_SPLIT) * 4 * 128 + j * 128
                        nc.tensor.matmul(
                            xh_ps[:, j, :],
                            h_sb[:, 0, e0 : e0 + 128],
                            xash[:, 0, :],
                            start=True,
                            stop=False,
                        )
                        nc.tensor.matmul(
                            xh_ps[:, j, :],
                            h_sb[:, 1, e0 : e0 + 128],
                            xash[:, 1, :],
                            start=False,
                            stop=True,
                        )
                    xh_sb = up.tile([128, 4, 128], BF16, tag="xhs")
                    nc.scalar.activation(
                        xh_sb[:].rearrange("p a f -> p (a f)"),
                        xh_ps[:].rearrange("p a f -> p (a f)"),
                        AF.Copy,
                    )
                    if b < 1:
                        g_sb = up.tile([128, 4, 128], BF16, tag="gsb")
                        nc.scalar.activation(
                            g_sb[:].rearrange("p a f -> p (a f)"),
                            g_ps[:].rearrange("p a f -> p (a f)"),
                            AF.Copy,
                        )
                        nc.vector.tensor_mul(
                            u[:].rearrange("p a f -> p (a f)"),
                            g_sb[:].rearrange("p a f -> p (a f)"),
                            xh_sb[:].rearrange("p a f -> p (a f)"),
                        )
                    else:
                        nc.vector.tensor_mul(
                            u[:].rearrange("p a f -> p (a f)"),
                            g_ps[:].rearrange("p a f -> p (a f)"),
                            xh_sb[:].rearrange("p a f -> p (a f)"),
                        )
                for j in range(4):
                    t = grp * 4 + j
                    # edge-tile t -> n-pair (2t, 2t+1); s-tile half = t // 64;
                    # 32-partition block bblk = (t % 64) // 16; slot r = t % 16.
                    # start/stop follow PROCESSING order (H-groups run first,
                    # so a block straddling the gather/H boundary starts at
                    # its first-processed tile).
                    half = t // 64
                    bblk = (t % 64) // 16
                    r = t % 16
                    blk0 = (t // 16) * 16
                    pos = [tile_pos[blk0 + rr] for rr in range(16)]
                    nc.tensor.matmul(
                        s_ps[32 * bblk : 32 * bblk + 32, half, :],
                        sel_sb[:, 32 - 2 * r : 64 - 2 * r],
                        u[:, j, :],
                        start=(tile_pos[t] == min(pos)),
                        stop=(tile_pos[t] == max(pos)),
                        tile_position=(0, 32 * bblk),
                    )
                # advance the previous batch's MLP chain a couple of ops
                if pending is not None:
                    for _ in range(2):
                        if next(pending, "done") == "done":
                            pending = None
                            break

            if pending is not None:
                for _ in pending:
                    pass
            pending = mlp_chain(b, P, xT, xi_sp, s_ps)

        if pending is not None:
            for _ in pending:
                pass

    nc.compile()
    return nc


def _prep_core_inputs(inputs):
    """Host-side layout prep. Returns in_maps for the 8 cores."""
    x = np.asarray(inputs["x"], np.float32)
    rbf = np.asarray(inputs["rbf"], np.float32)
    neighbor = np.asarray(inputs["neighbor"])
    k2f_W = np.asarray(inputs["k2f_W"], np.float32)

    c = LN2

    def lhsT(w):
        return np.ascontiguousarray(np.asarray(w, np.float32).T)

    # weight stack [F, 12, F]
    ws = np.zeros((F, 12, F), np.float32)
    ws[:, IW_WI, :] = lhsT(inputs["Wi"])
    for l in range(3):
        ws[:, IW_IRES + 2 * l, :] = lhsT(inputs["ires_W1"][l])
        ws[:, IW_IRES + 2 * l + 1, :] = lhsT(inputs["ires_W2"][l])
    ws[:, IW_WINT, :] = lhsT(inputs["Wint"])
    for l in range(2):
        ws[:, IW_ARES + 2 * l, :] = lhsT(inputs["ares_W1"][l])
        ws[:, IW_ARES + 2 * l + 1, :] = lhsT(inputs["ares_W2"][l])

    # bias stack [F, NB] (softplus shift folded in)
    bs = np.zeros((F, NB), np.float32)
    rs = lambda w: np.asarray(w, np.float32).sum(axis=1)
    bs[:, IB_NLN2] = -c
    bs[:, IB_HALF] = 0.5
    bs[:, IB_UGATE] = np.asarray(inputs["u_gate"], np.float32)
    bs[:, IB_WI] = inputs["bi"] - c * rs(inputs["Wi"])
    for l in range(3):
        bs[:, IB_IRES1 + l] = inputs["ires_b1"][l] - c * rs(inputs["ires_W1"][l])
        bs[:, IB_IRES2 + l] = inputs["ires_b2"][l] - c * rs(inputs["ires_W2"][l])
    bs[:, IB_WINT] = inputs["bint"] - c * rs(inputs["Wint"])
    for l in range(2):
        bs[:, IB_ARES1 + l] = inputs["ares_b1"][l] - c * rs(inputs["ares_W1"][l])
        bs[:, IB_ARES2 + l] = inputs["ares_b2"][l] - c * rs(inputs["ares_W2"][l])

    k2fT = np.ascontiguousarray(k2f_W.T).astype(ml_dtypes.bfloat16)  # [K, F]

    selbuf = np.zeros((128, 66), ml_dtypes.bfloat16)
    selbuf[:64, 32] = 1
    selbuf[64:, 33] = 1

    ident = np.eye(128, dtype=np.float32)

    in_maps = []
    eye256 = np.eye(256, dtype=ml_dtypes.float8_e4m3fn)
    for i in range(NCORES):
        bs_lo = i * BPC
        x_c = np.ascontiguousarray(x[bs_lo : bs_lo + BPC])
        rbf_c = rbf[bs_lo : bs_lo + BPC].reshape(BPC * E_B, K)
        rbfT_c = np.ascontiguousarray(rbf_c.T).astype(ml_dtypes.bfloat16)
        nbr_c = neighbor[bs_lo : bs_lo + BPC].reshape(BPC, E_B).astype(np.int64)
        # gather idxs: first E_G edges of each batch.
        # dma_gather wrap: idx i -> partition i%16, col i//16, per call
        idx_c = np.zeros((128, BPC * E_G // 16), np.int16)
        # one-hot H for the last E_H edges: [BPC, 2, 128, E_H]
        h_c = np.zeros((BPC, 2, 128, E_H), ml_dtypes.float8_e4m3fn)
        for b in range(BPC):
            nb_b = nbr_c[b]
            ng = nb_b[:E_G].astype(np.int16)
            off = 0
            for gsz in G_SIZES:
                seg = ng[off : off + gsz]
                wrap = np.tile(seg.reshape(gsz // 16, 16).T, (8, 1))
                col0 = b * (E_G // 16) + off // 16
                idx_c[:, col0 : col0 + gsz // 16] = wrap
                off += gsz
            nh = nb_b[E_G:]
            h_b = eye256[:, nh]  # [256, E_H] one-hot columns
            h_c[b, 0] = h_b[:128]
            h_c[b, 1] = h_b[128:]
        in_maps.append(
            {
                "x": x_c,
                "rbfT": rbfT_c,
                "idx": idx_c,
                "hmat": h_c,
                "wstack": ws,
                "bstack": bs,
                "k2fT": k2fT,
                "selbuf": selbuf,
                "ident": ident,
            }
        )
    return in_maps


def run(inputs, trace=False, **kwargs):
    global _GRAPH
    if _GRAPH is None:
        _GRAPH = build_graph()
    in_maps = _prep_core_inputs(inputs)
    res = run_bass_kernel_spmd(
        _GRAPH, in_maps, core_ids=list(range(NCORES)), trace=trace, **kwargs
    )
    outs = [np.asarray(res.results[i]["out"], np.float32) for i in range(NCORES)]
    full = np.concatenate(outs, axis=0)  # [B, N, F]
    return full, res


def kernel(**inputs):
    full, _ = run(inputs, trace=False)
    return full
